# revision 2
# baseline (speedup 1.0000x reference)
"""Trainium2 Bass kernel for CrossAttention (LN -> QKV proj -> MHA -> out proj).

Sharding: data-parallel over (batch, query-half): 8 shards for B=4.
Each core gets a [1024, 1024] query-token slice and the full [2048, 768]
context for its batch, and produces a [1024, 1024] output slice.

Per-core dataflow (matmul operands bf16, accumulation fp32 in PSUM):
  - LayerNorm on query/context tokens in natural [tok, C] layout (DVE
    bn_stats / bn_aggr), gamma/beta applied with partition-broadcast rows.
  - Transpose LN'd activations to channel-major [C, tok] via DMA xbar
    transpose (bf16) so channels sit on the contraction (partition) axis.
    Activations/projections are chunked (512 tokens) so projections start
    while later chunks are still normalizing.
  - QT = Wq^T @ XqT, KT = Wk^T @ XcT (channel-major), V natural [tok, C].
  - Attention runs per head-quad: scoresT[k, q] = K_h @ Q_h^T (contraction
    D=64; even/odd heads at partitions 0-63/64-127 land on disjoint PE row
    groups and overlap), exp on ACT straight out of PSUM (scores are
    bounded, so no max subtraction).  attendedT accumulates per head pair
    into one [128, 512] psum via column tiling (head0 rows 0-63, head1
    rows 64-127, concurrent).  Softmax denominators come from ones-vector
    matmuls, 4 heads packed at output partitions 0/32/64/96 of one psum.
  - Normalize with DVE reciprocal + DRAM-bounce partition broadcast, then
    out = attendedT^T @ Wo + bo in natural layout, DMA out.
"""

import numpy as np

import concourse.bass as bass
import concourse.tile as tile
from concourse import mybir
from concourse.bass_utils import run_bass_kernel_spmd

F32 = mybir.dt.float32
BF16 = mybir.dt.bfloat16
AF = mybir.ActivationFunctionType
OP = mybir.AluOpType

B, NQ_FULL, NK, CQ, CK, H, D = 4, 2048, 2048, 1024, 768, 16, 64
NQ = 1024            # per-core query tokens
N_CORES = 8
EPS = 1e-5
SM_SCALE = 1.0 / np.sqrt(D)  # 0.125

KC_Q = CQ // 128     # 8  contraction chunks for CQ
KC_C = CK // 128     # 6  contraction chunks for CK
NQT = NQ // 128      # 8  query token tiles
NKT = NK // 128      # 16 context token tiles
QC = 512             # q processed in chunks of 512 (psum free-dim limit)
NQ2 = NQ // QC       # 2


def _split_excess_waits(nc, max_waits=1):
    """walrus in this container accepts at most one sync wait per
    instruction; Tile's kernel-tail drain carries several.  Hoist excess
    waits onto single-wait NOPs that precede the instruction on the same
    engine (absolute sem waits commute, so this is semantics-preserving)."""
    for fn in nc.m.functions:
        for blk in fn.blocks:
            out = []
            dirty = False
            for inst in list(blk.instructions):
                si = inst.sync_info
                if si is not None and len(si.on_wait) > max_waits:
                    waits = list(si.on_wait)
                    for k, w in enumerate(waits[:-max_waits]):
                        nop = mybir.InstNoOp(
                            name=f"wsplit-{inst.name}-{k}", ins=[], outs=[])
                        nop.engine = inst.engine
                        nop.sync_info = mybir.SyncInfo(on_wait=[w], on_update=[])
                        out.append(nop)
                    inst.sync_info = mybir.SyncInfo(
                        on_wait=waits[-max_waits:], on_update=list(si.on_update))
                    dirty = True
                out.append(inst)
            if dirty:
                blk.instructions = out


def _bcast_ap(handle, n_parts, n_free):
    """DRAM [n_free] vector replicated across n_parts partitions."""
    return bass.AP(tensor=handle.ap().tensor, offset=0,
                   ap=[[0, n_parts], [1, n_free]])


def _emit(tc, t, out, stages=("proj", "attn", "out")):
    from contextlib import ExitStack
    nc = tc.nc

    es = ExitStack()
    persist = es.enter_context(tc.tile_pool(name="persist", bufs=1))

    # chunked persistent tensors (distinct tags => distinct slots)
    # kT split per (channel-chunk, token-chunk): finer deps let attention
    # start while later context chunks are still projecting
    qTc = [persist.tile([128, NQ], BF16, tag=f"qT{oc}", name=f"qT{oc}")
           for oc in range(KC_Q)]
    kTc = [[persist.tile([128, QC], BF16, tag=f"kT{oc}_{t4}",
                         name=f"kT{oc}_{t4}") for t4 in range(NK // QC)]
           for oc in range(KC_Q)]
    # V with a ones column appended per head: the attended-value matmul
    # then also emits the softmax denominator (row 64 of its psum)
    v_g = [persist.tile([128, 4, H, D + 1], BF16, tag=f"v{g}", name=f"v{g}")
           for g in range(NKT // 4)]
    bq_cols = persist.tile([128, KC_Q], F32)
    bk_cols = persist.tile([128, KC_Q], F32)
    bvb = persist.tile([128, CQ], F32)
    eps_t = persist.tile([128, 1], F32)

    nc.vector.memset(eps_t[:, :], EPS)
    nc.scalar.dma_start(out=bq_cols[:, :],
                        in_=t["bq"].ap().rearrange("(j p) -> p j", p=128))
    nc.scalar.dma_start(out=bk_cols[:, :],
                        in_=t["bk"].ap().rearrange("(j p) -> p j", p=128))
    nc.gpsimd.dma_start(out=bvb[:, :], in_=_bcast_ap(t["bv"], 128, CQ))

    # ---------------- phase 1+2: LN, transpose, projections ----------------
    with tc.tile_pool(name="pps", bufs=3, space="PSUM") as pps, \
         tc.tile_pool(name="lnw", bufs=1) as lnw, \
         tc.tile_pool(name="xfp", bufs=3) as xfp, \
         tc.tile_pool(name="stp", bufs=4) as stp, \
         tc.tile_pool(name="bfp", bufs=4) as bfp, \
         tc.tile_pool(name="wfp", bufs=3) as wfp, \
         tc.tile_pool(name="xTp", bufs=2) as xTp, \
         tc.tile_pool(name="wbp", bufs=1) as wbp:

        gqb = lnw.tile([128, CQ], F32)
        bqb = lnw.tile([128, CQ], F32)
        gcb = lnw.tile([128, CK], F32)
        bcb = lnw.tile([128, CK], F32)
        nc.gpsimd.dma_start(out=gqb[:, :], in_=_bcast_ap(t["gamma_q"], 128, CQ))
        nc.gpsimd.dma_start(out=bqb[:, :], in_=_bcast_ap(t["beta_q"], 128, CQ))
        nc.gpsimd.dma_start(out=gcb[:, :], in_=_bcast_ap(t["gamma_ctx"], 128, CK))
        nc.gpsimd.dma_start(out=bcb[:, :], in_=_bcast_ap(t["beta_ctx"], 128, CK))

        def ln_tile(x_dram, i, C, n_sub, sub, gb, bb, xT_chunk, col0):
            """LN one [128, C] token tile, write bf16 transpose into
            xT_chunk[kc][:, col0:col0+128] for each channel chunk kc."""
            xf = xfp.tile([128, C], F32, tag="xf", name=f"xf_{i}_{C}")
            nc.scalar.dma_start(out=xf[:, :],
                                in_=x_dram.ap()[i * 128:(i + 1) * 128, :])
            st = stp.tile([128, n_sub, 6], F32, tag="st", name=f"st_{i}_{C}")
            for s in range(n_sub):
                nc.vector.bn_stats(out=st[:, s, :],
                                   in_=xf[:, s * sub:(s + 1) * sub])
            mv = stp.tile([128, 2], F32, tag="mv", name=f"mv_{i}_{C}")
            nc.vector.bn_aggr(out=mv[:, :], in_=st[:, :, :])
            nc.scalar.activation(out=mv[:, 1:2], in_=mv[:, 1:2],
                                 func=AF.Sqrt, bias=eps_t[:, :], scale=1.0)
            nc.vector.reciprocal(out=mv[:, 1:2], in_=mv[:, 1:2])
            nc.vector.tensor_scalar(out=xf[:, :], in0=xf[:, :],
                                    scalar1=mv[:, 0:1], scalar2=mv[:, 1:2],
                                    op0=OP.subtract, op1=OP.mult)
            nc.vector.tensor_mul(out=xf[:, :], in0=xf[:, :], in1=gb[:, :])
            xbf = bfp.tile([128, C], BF16, tag="xbf", name=f"xbf_{i}_{C}")
            nc.vector.tensor_add(out=xbf[:, :], in0=xf[:, :], in1=bb[:, :])
            for j in range(C // 128):
                nc.sync.dma_start(out=xT_chunk[j][:, col0:col0 + 128],
                                  in_=xbf[:, j * 128:(j + 1) * 128],
                                  transpose=True)

        def load_w(dram, n_chunks, tagp):
            tiles = []
            for kc in range(n_chunks):
                wf = wfp.tile([128, CQ], F32, tag="wf", name=f"wf{tagp}{kc}")
                nc.scalar.dma_start(out=wf[:, :],
                                    in_=dram.ap()[kc * 128:(kc + 1) * 128, :])
                wb = wbp.tile([128, CQ], BF16, tag=f"w{tagp}{kc}",
                              name=f"w{tagp}{kc}")
                nc.gpsimd.tensor_copy(out=wb[:, :], in_=wf[:, :])
                tiles.append(wb)
            return tiles

        # ---- query side, chunked by 512 tokens ----
        wq = load_w(t["Wq"], KC_Q, "q")
        for t2 in range(NQ2):
            xqT = [xTp.tile([128, QC], BF16, tag=f"xqT{kc}",
                            name=f"xqT{kc}_{t2}") for kc in range(KC_Q)]
            for i in range(4):
                ln_tile(t["xq"], t2 * 4 + i, CQ, 2, 512, gqb, bqb,
                        xqT, i * 128)
            for oc in range(KC_Q):
                ps = pps.tile([128, QC], F32, tag="pp", name=f"psq{oc}_{t2}")
                for kc in range(KC_Q):
                    nc.tensor.matmul(ps[:, :],
                                     wq[kc][:, oc * 128:(oc + 1) * 128],
                                     xqT[kc][:, :],
                                     start=(kc == 0), stop=(kc == KC_Q - 1))
                nc.scalar.activation(
                    out=qTc[oc][:, t2 * QC:(t2 + 1) * QC], in_=ps[:, :],
                    func=AF.Identity, bias=bq_cols[:, oc:oc + 1], scale=1.0)

        # ---- context side, chunked by 512 tokens ----
        wk = load_w(t["Wk"], KC_C, "k")
        wv = load_w(t["Wv"], KC_C, "v")
        for t4 in range(NK // QC):
            xcT = [xTp.tile([128, QC], BF16, tag=f"xcT{kc}",
                            name=f"xcT{kc}_{t4}") for kc in range(KC_C)]
            for i in range(4):
                ln_tile(t["xc"], t4 * 4 + i, CK, 3, 256, gcb, bcb,
                        xcT, i * 128)
            for oc in range(KC_Q):
                ps = pps.tile([128, QC], F32, tag="pp", name=f"psk{oc}_{t4}")
                for kc in range(KC_C):
                    nc.tensor.matmul(ps[:, :],
                                     wk[kc][:, oc * 128:(oc + 1) * 128],
                                     xcT[kc][:, :],
                                     start=(kc == 0), stop=(kc == KC_C - 1))
                nc.scalar.activation(
                    out=kTc[oc][t4][:, :], in_=ps[:, :], func=AF.Identity,
                    bias=bk_cols[:, oc:oc + 1], scale=1.0)
            for ki in range(4):
                kt = t4 * 4 + ki
                for v2 in range(CQ // QC):
                    ps = pps.tile([128, QC], F32, tag="pp",
                                  name=f"psv{kt}_{v2}")
                    for kc in range(KC_C):
                        nc.tensor.matmul(ps[:, :],
                                         xcT[kc][:, ki * 128:(ki + 1) * 128],
                                         wv[kc][:, v2 * QC:(v2 + 1) * QC],
                                         start=(kc == 0), stop=(kc == KC_C - 1))
                    nc.vector.tensor_tensor(
                        out=v_g[t4][:, ki, v2 * 8:(v2 + 1) * 8, 0:D],
                        in0=ps[:, :].rearrange("p (h d) -> p h d", d=D),
                        in1=bvb[:, v2 * QC:(v2 + 1) * QC].rearrange(
                            "p (h d) -> p h d", d=D),
                        op=OP.add)
                nc.vector.memset(v_g[t4][:, ki, :, D:D + 1], 1.0)

    # ---------------- phase 3: attention ----------------
    if "attn" not in stages:
        # timing-only partial build: flush something derived to out
        with tc.tile_pool(name="fl", bufs=1) as fl:
            fb = fl.tile([128, QC], F32, name="fb")
            nc.vector.tensor_copy(out=fb[:, :], in_=qTc[0][:, 0:QC])
            nc.sync.dma_start(out=out.ap()[0:128, 0:QC], in_=fb[:, :])
        es.close()
        return
    late = es.enter_context(tc.tile_pool(name="late", bufs=1))
    attT = late.tile([128, KC_Q, NQ], BF16, name="attT")
    wo = late.tile([128, KC_Q, CQ], BF16, name="wo")
    bob = late.tile([128, CQ], F32, name="bob")

    with tc.tile_pool(name="scps", bufs=2, space="PSUM") as scps, \
         tc.tile_pool(name="attps", bufs=2, space="PSUM") as attps, \
         tc.tile_pool(name="ep", bufs=4) as ep, \
         tc.tile_pool(name="rp", bufs=4) as rp, \
         tc.tile_pool(name="tmp1", bufs=2) as tmp1p, \
         tc.tile_pool(name="scr", bufs=4, space="DRAM") as scr, \
         tc.tile_pool(name="wfp2", bufs=2) as wfp2:

        nc.gpsimd.dma_start(out=bob[:, :], in_=_bcast_ap(t["bo"], 128, CQ))
        for kc in range(KC_Q):
            wof = wfp2.tile([128, CQ], F32, tag="wof", name=f"wof{kc}")
            nc.scalar.dma_start(out=wof[:, :],
                                in_=t["Wo"].ap()[kc * 128:(kc + 1) * 128, :])
            nc.gpsimd.tensor_copy(out=wo[:, kc, :], in_=wof[:, :])

        for hp in range(H // 2):
            att = {}
            for par in range(2):
                h = 2 * hp + par
                att[par] = attps.tile([D + 1, NQ], F32, tag="att",
                                      name=f"attp{h}")
            for kt in range(NKT):
                for par in range(2):
                    h, lo = 2 * hp + par, par * 64
                    sc = scps.tile([128, NQ], F32, tag="sc",
                                   name=f"sc{h}_{kt}")
                    for q2 in range(NQ2):
                        nc.tensor.matmul(
                            sc[:, q2 * QC:(q2 + 1) * QC],
                            kTc[hp][kt // 4][lo:lo + 64,
                                             (kt % 4) * 128:(kt % 4 + 1) * 128],
                            qTc[hp][lo:lo + 64, q2 * QC:(q2 + 1) * QC],
                            start=True, stop=True)
                    # one exp over the full q width (both psum banks):
                    # halves the per-instruction ACT overhead
                    e = ep.tile([128, NQ], BF16, tag="e", name=f"e{h}_{kt}")
                    nc.scalar.activation(out=e[:, :], in_=sc[:, :],
                                         func=AF.Exp, scale=SM_SCALE)
                    # attended + softmax denominator in one matmul:
                    # lhsT = [V_h | ones], row 64 of psum = sum(exp)
                    for q2 in range(NQ2):
                        nc.tensor.matmul(
                            att[par][:, q2 * QC:(q2 + 1) * QC],
                            v_g[kt // 4][:, kt % 4, h, :],
                            e[:, q2 * QC:(q2 + 1) * QC],
                            start=(kt == 0), stop=(kt == NKT - 1))
            for par in range(2):
                h = 2 * hp + par
                # drain psum to SBUF right away so the accumulator slot
                # frees for the next head pair; the (slow) normalize chain
                # then runs off the SBUF copy, off the critical path
                atc = rp.tile([64, NQ], F32, tag="atc", name=f"atc{h}")
                nc.vector.tensor_copy(out=atc[:, :], in_=att[par][0:D, :])
                rec = rp.tile([65, NQ], F32, tag="rec", name=f"rec{h}")
                nc.vector.reciprocal(out=rec[64:65, :], in_=att[par][64:65, :])
                sd = scr.tile([1, NQ], F32, tag="sd", name=f"sd{h}")
                nc.sync.dma_start(out=sd[:, :], in_=rec[64:65, :])
                rb = rp.tile([64, NQ], F32, tag="rb", name=f"rb{h}")
                nc.sync.dma_start(
                    out=rb[:, :],
                    in_=bass.AP(tensor=sd.tensor, offset=sd.offset,
                                ap=[[0, 64], [1, NQ]]))
                if par == 0:
                    nc.vector.tensor_mul(out=attT[0:64, hp, :],
                                         in0=atc[:, :], in1=rb[:, :])
                else:
                    # odd head: normalize at partitions 0-63, then DMA
                    # shifts it to partitions 64-127 of the attT chunk
                    tm = tmp1p.tile([64, NQ], BF16, tag="tm", name=f"tm{h}")
                    nc.vector.tensor_mul(out=tm[:, :],
                                         in0=atc[:, :], in1=rb[:, :])
                    nc.sync.dma_start(out=attT[64:128, hp, :], in_=tm[:, :])

    # ---------------- phase 4: out projection ----------------
    with tc.tile_pool(name="ops", bufs=2, space="PSUM") as ops, \
         tc.tile_pool(name="op", bufs=2) as op_pool:
        if "out" not in stages:
            fb2 = op_pool.tile([128, QC], F32, name="fb2")
            nc.vector.tensor_copy(out=fb2[:, :], in_=attT[:, 0, 0:QC])
            nc.sync.dma_start(out=out.ap()[0:128, 0:QC], in_=fb2[:, :])
        for qt in range(NQT if "out" in stages else 0):
            osb = op_pool.tile([128, CQ], F32, tag="osb", name=f"osb{qt}")
            for cc in range(CQ // QC):
                ps = ops.tile([128, QC], F32, tag="opp", name=f"pso{qt}_{cc}")
                for kc in range(KC_Q):
                    nc.tensor.matmul(
                        ps[:, :],
                        attT[:, kc, qt * 128:(qt + 1) * 128],
                        wo[:, kc, cc * QC:(cc + 1) * QC],
                        start=(kc == 0), stop=(kc == KC_Q - 1))
                nc.vector.tensor_tensor(out=osb[:, cc * QC:(cc + 1) * QC],
                                        in0=ps[:, :],
                                        in1=bob[:, cc * QC:(cc + 1) * QC],
                                        op=OP.add)
            nc.sync.dma_start(out=out.ap()[qt * 128:(qt + 1) * 128, :],
                              in_=osb[:, :])

    es.close()


def build():
    nc = bass.Bass("TRN2", target_bir_lowering=False, debug=False,
                   num_devices=N_CORES)
    t = {
        "xq": nc.dram_tensor("xq", [NQ, CQ], F32, kind="ExternalInput"),
        "xc": nc.dram_tensor("xc", [NK, CK], F32, kind="ExternalInput"),
        "Wq": nc.dram_tensor("Wq", [CQ, CQ], F32, kind="ExternalInput"),
        "Wk": nc.dram_tensor("Wk", [CK, CQ], F32, kind="ExternalInput"),
        "Wv": nc.dram_tensor("Wv", [CK, CQ], F32, kind="ExternalInput"),
        "Wo": nc.dram_tensor("Wo", [CQ, CQ], F32, kind="ExternalInput"),
        "bq": nc.dram_tensor("bq", [CQ], F32, kind="ExternalInput"),
        "bk": nc.dram_tensor("bk", [CQ], F32, kind="ExternalInput"),
        "bv": nc.dram_tensor("bv", [CQ], F32, kind="ExternalInput"),
        "bo": nc.dram_tensor("bo", [CQ], F32, kind="ExternalInput"),
        "gamma_q": nc.dram_tensor("gamma_q", [CQ], F32, kind="ExternalInput"),
        "beta_q": nc.dram_tensor("beta_q", [CQ], F32, kind="ExternalInput"),
        "gamma_ctx": nc.dram_tensor("gamma_ctx", [CK], F32, kind="ExternalInput"),
        "beta_ctx": nc.dram_tensor("beta_ctx", [CK], F32, kind="ExternalInput"),
    }
    out = nc.dram_tensor("out", [NQ, CQ], F32, kind="ExternalOutput")
    with tile.TileContext(nc) as tc:
        _emit(tc, t, out)
    _split_excess_waits(nc)
    return nc


_NC = None


def _in_maps(inputs):
    q = np.ascontiguousarray(np.asarray(inputs["query_tokens"], dtype=np.float32))
    c = np.ascontiguousarray(np.asarray(inputs["context_tokens"], dtype=np.float32))
    shared = {k: np.ascontiguousarray(np.asarray(inputs[k], dtype=np.float32))
              for k in ("Wq", "Wk", "Wv", "Wo", "bq", "bk", "bv", "bo",
                        "gamma_q", "beta_q", "gamma_ctx", "beta_ctx")}
    maps = []
    for core in range(N_CORES):
        b, half = core // 2, core % 2
        m = dict(shared)
        m["xq"] = np.ascontiguousarray(q[b, half * NQ:(half + 1) * NQ, :])
        m["xc"] = np.ascontiguousarray(c[b])
        maps.append(m)
    return maps


def run_sharded(inputs, **kwargs):
    global _NC
    if _NC is None:
        _NC = build()
    return run_bass_kernel_spmd(_NC, _in_maps(inputs),
                                core_ids=list(range(N_CORES)), **kwargs)


def assemble(res) -> np.ndarray:
    out = np.empty((B, NQ_FULL, CQ), np.float32)
    for core in range(N_CORES):
        b, half = core // 2, core % 2
        out[b, half * NQ:(half + 1) * NQ, :] = res.results[core]["out"]
    return out


def kernel(**inputs) -> np.ndarray:
    return assemble(run_sharded(inputs))



# revision 10
# speedup vs baseline: 1.4678x; 1.4678x over previous
"""Trainium2 Bass kernel for CrossAttention (LN -> QKV proj -> MHA -> out proj).

Sharding: data-parallel over (batch, query-half): 8 shards for B=4.
Each core gets a [1024, 1024] query-token slice and the full [2048, 768]
context for its batch, and produces a [1024, 1024] output slice.

Per-core dataflow (matmul operands bf16, accumulation fp32 in PSUM):
  - LayerNorm in natural [tok, C] layout (DVE bn_stats/bn_aggr); the
    normalized bf16 activations bounce through a DRAM scratch so the
    channel-major transpose runs as a few large DMA-xbar transposes
    ([512 tok, 128 ch] each) instead of many [128,128] ones.
  - Weights are cast fp32->bf16 during the load DMA (SWDGE).
  - Context side (K/V) is projected first so attention can start while
    the query-side projection passes still run under it.
  - Attention per head-pair hp, query-half qh (512 q), context tile kt:
    the two heads' score matmuls have contraction D=64 at disjoint PE
    row groups (partitions 0-63 / 64-127) with identical deps, so they
    execute concurrently in the array; one exp (ACT) covers both heads'
    scores [128, 2*512] straight out of PSUM.  attended accumulates per
    (hp, qh, par) into a [65, 512] psum over all kt, with a ones column
    appended to V so row 64 yields the softmax denominator for free.
  - Denominators of the 4 (head, qh) rows of a pair are batched through
    a DRAM scratch into a [16, 128] tile for one partition-parallel
    fast reciprocal, then broadcast-DMA'd back to [64, 1024] rows for
    the normalize multiply (DVE), off the attention critical path.
  - out = attT^T @ Wo + bo per 128-token tile, DMA out.

PSUM budget: 2 banks proj/out + 2x2 banks scores + 2 banks attended = 8.
SBUF: persistent (qTc/kTc/v_g/attT ~100KB/part) + phase-scoped pools.
"""

import numpy as np

import concourse.bass as bass
import concourse.tile as tile
from concourse import mybir
from concourse.bass_utils import run_bass_kernel_spmd

F32 = mybir.dt.float32
BF16 = mybir.dt.bfloat16
AF = mybir.ActivationFunctionType
OP = mybir.AluOpType

B, NQ_FULL, NK, CQ, CK, H, D = 4, 2048, 2048, 1024, 768, 16, 64
NQ = 1024            # per-core query tokens
N_CORES = 8
EPS = 1e-5
SM_SCALE = 1.0 / np.sqrt(D)  # 0.125

KC_Q = CQ // 128     # 8  channel chunks for CQ
KC_C = CK // 128     # 6  channel chunks for CK
NQT = NQ // 128      # 8  query token tiles
NKT = NK // 128      # 16 context token tiles
QC = 512             # psum free-dim limit (fp32)
T4 = NK // QC        # 4  context 512-token chunks
NQ2 = NQ // QC       # 2  query 512-token chunks


def _split_excess_waits(nc, max_waits=1):
    """walrus in this container accepts at most one sync wait per
    instruction; Tile's kernel-tail drain carries several.  Hoist excess
    waits onto single-wait NOPs that precede the instruction on the same
    engine (absolute sem waits commute, so this is semantics-preserving)."""
    for fn in nc.m.functions:
        for blk in fn.blocks:
            out = []
            dirty = False
            for inst in list(blk.instructions):
                si = inst.sync_info
                if si is not None and len(si.on_wait) > max_waits:
                    waits = list(si.on_wait)
                    for k, w in enumerate(waits[:-max_waits]):
                        nop = mybir.InstNoOp(
                            name=f"wsplit-{inst.name}-{k}", ins=[], outs=[])
                        nop.engine = inst.engine
                        nop.sync_info = mybir.SyncInfo(on_wait=[w], on_update=[])
                        out.append(nop)
                    inst.sync_info = mybir.SyncInfo(
                        on_wait=waits[-max_waits:], on_update=list(si.on_update))
                    dirty = True
                out.append(inst)
            if dirty:
                blk.instructions = out


def _bcast_ap(handle, n_parts, n_free):
    """DRAM [n_free] vector replicated across n_parts partitions."""
    return bass.AP(tensor=handle.ap().tensor, offset=0,
                   ap=[[0, n_parts], [1, n_free]])


def _emit(tc, t, out):
    from contextlib import ExitStack
    nc = tc.nc

    es = ExitStack()
    persist = es.enter_context(tc.tile_pool(name="persist", bufs=1))
    dram = es.enter_context(tc.tile_pool(name="dram", bufs=1, space="DRAM"))

    # ---- tensors live across phases ----
    qTc = [persist.tile([128, NQ], BF16, tag=f"qT{oc}", name=f"qT{oc}")
           for oc in range(KC_Q)]
    kTc = [[persist.tile([128, QC], BF16, tag=f"kT{oc}_{t4}",
                         name=f"kT{oc}_{t4}") for t4 in range(T4)]
           for oc in range(KC_Q)]
    # V with a ones column per head: attended matmul row 64 = sum(exp)
    v_g = [persist.tile([128, 4, H, D + 1], BF16, tag=f"v{g}", name=f"v{g}")
           for g in range(T4)]
    bq_cols = persist.tile([128, KC_Q], F32)
    bk_cols = persist.tile([128, KC_Q], F32)
    bob = persist.tile([128, CQ], F32)
    eps_t = persist.tile([128, 1], F32)

    # DRAM scratch
    xq_bf = dram.tile([NQ, CQ], BF16, name="xq_bf")
    xc_bf = dram.tile([NK, CK], BF16, name="xc_bf")
    den_d = dram.tile([1, H * NQ], BF16, name="den_d")
    denr_d = dram.tile([1, H * NQ], F32, name="denr_d")

    # ---- PSUM pools: 2 (proj/out) + 4 (scores x2bufs) + 2 (attended) = 8
    pps = es.enter_context(tc.tile_pool(name="pps", bufs=2, space="PSUM"))
    scps = es.enter_context(tc.tile_pool(name="scps", bufs=2, space="PSUM"))
    attps = es.enter_context(tc.tile_pool(name="attps", bufs=1, space="PSUM"))

    # e tiles sit on attention's critical path at the phase boundary, so
    # they get addresses disjoint from the phase-A pools below.
    ep = es.enter_context(tc.tile_pool(name="ep", bufs=3))

    nc.vector.memset(eps_t[:, :], EPS)
    nc.sync.dma_start(out=bq_cols[:, :],
                      in_=t["bq"].ap().rearrange("(j p) -> p j", p=128))
    nc.sync.dma_start(out=bk_cols[:, :],
                      in_=t["bk"].ap().rearrange("(j p) -> p j", p=128))
    nc.gpsimd.dma_start(out=bob[:, :], in_=_bcast_ap(t["bo"], 128, CQ))

    # =================== phase A: LN + QKV projections ===================
    with tc.tile_pool(name="wp", bufs=1) as wp, \
         tc.tile_pool(name="lnc", bufs=1) as lnc, \
         tc.tile_pool(name="xfp", bufs=3) as xfp, \
         tc.tile_pool(name="stp", bufs=3) as stp, \
         tc.tile_pool(name="xTq", bufs=1) as xTq, \
         tc.tile_pool(name="xTc", bufs=1) as xTc:

        # weights: fp32 DRAM -> bf16 SBUF, cast during SWDGE DMA
        wq = [wp.tile([128, CQ], BF16, tag=f"wq{k}", name=f"wq{k}")
              for k in range(KC_Q)]
        wk = [wp.tile([128, CQ], BF16, tag=f"wk{k}", name=f"wk{k}")
              for k in range(KC_C)]
        wv = [wp.tile([128, CQ], BF16, tag=f"wv{k}", name=f"wv{k}")
              for k in range(KC_C)]
        for kc in range(KC_C):
            nc.gpsimd.dma_start(out=wk[kc][:, :],
                                in_=t["Wk"].ap()[kc * 128:(kc + 1) * 128, :])
            nc.gpsimd.dma_start(out=wv[kc][:, :],
                                in_=t["Wv"].ap()[kc * 128:(kc + 1) * 128, :])
        gqb = lnc.tile([128, CQ], F32)
        bqb = lnc.tile([128, CQ], F32)
        gcb = lnc.tile([128, CK], F32)
        bcb = lnc.tile([128, CK], F32)
        bvb = lnc.tile([128, CQ], F32)
        nc.gpsimd.dma_start(out=gqb[:, :], in_=_bcast_ap(t["gamma_q"], 128, CQ))
        nc.gpsimd.dma_start(out=bqb[:, :], in_=_bcast_ap(t["beta_q"], 128, CQ))
        nc.gpsimd.dma_start(out=gcb[:, :], in_=_bcast_ap(t["gamma_ctx"], 128, CK))
        nc.gpsimd.dma_start(out=bcb[:, :], in_=_bcast_ap(t["beta_ctx"], 128, CK))
        nc.gpsimd.dma_start(out=bvb[:, :], in_=_bcast_ap(t["bv"], 128, CQ))

        def ln_tile(x_dram, bf_dram, i, C, n_sub, sub, gb, bb):
            """LN one [128, C] token tile in natural layout (bf16 in via
            cast-DMA); bf16 result goes to the DRAM bounce buffer for the
            batched transpose."""
            xf = xfp.tile([128, CQ], BF16, tag="xf", name=f"xf_{i}_{C}")
            nc.gpsimd.dma_start(out=xf[:, 0:C],
                                in_=x_dram.ap()[i * 128:(i + 1) * 128, :])
            st = stp.tile([128, n_sub, 6], F32, tag="st", name=f"st_{i}_{C}")
            for s in range(n_sub):
                nc.vector.bn_stats(out=st[:, s, :],
                                   in_=xf[:, s * sub:(s + 1) * sub])
            mv = stp.tile([128, 2], F32, tag="mv", name=f"mv_{i}_{C}")
            nc.vector.bn_aggr(out=mv[:, :], in_=st[:, :, :])
            nc.scalar.activation(out=mv[:, 1:2], in_=mv[:, 1:2],
                                 func=AF.Sqrt, bias=eps_t[:, :], scale=1.0)
            nc.vector.reciprocal(out=mv[:, 1:2], in_=mv[:, 1:2])
            nc.vector.tensor_scalar(out=xf[:, 0:C], in0=xf[:, 0:C],
                                    scalar1=mv[:, 0:1], scalar2=mv[:, 1:2],
                                    op0=OP.subtract, op1=OP.mult)
            nc.vector.tensor_mul(out=xf[:, 0:C], in0=xf[:, 0:C], in1=gb[:, :])
            nc.vector.tensor_add(out=xf[:, 0:C], in0=xf[:, 0:C], in1=bb[:, :])
            nc.sync.dma_start(out=bf_dram[i * 128:(i + 1) * 128, :],
                              in_=xf[:, 0:C])

        # ---- context side: LN, transpose, K/V projections ----
        for t4 in range(T4):
            for i in range(4):
                ln_tile(t["xc"], xc_bf, t4 * 4 + i, CK, 3, 256, gcb, bcb)
            xcT = [xTc.tile([128, QC], BF16, tag=f"xcT{kc}",
                            name=f"xcT{kc}_{t4}") for kc in range(KC_C)]
            for kc in range(KC_C):
                nc.sync.dma_start(
                    out=xcT[kc][:, :],
                    in_=xc_bf[t4 * QC:(t4 + 1) * QC,
                              kc * 128:(kc + 1) * 128],
                    transpose=True)
            for oc in range(KC_Q):
                ps = pps.tile([128, QC], F32, tag="pp", name=f"psk{oc}_{t4}")
                for kc in range(KC_C):
                    nc.tensor.matmul(ps[:, :],
                                     wk[kc][:, oc * 128:(oc + 1) * 128],
                                     xcT[kc][:, :],
                                     start=(kc == 0), stop=(kc == KC_C - 1))
                nc.scalar.activation(
                    out=kTc[oc][t4][:, :], in_=ps[:, :], func=AF.Identity,
                    bias=bk_cols[:, oc:oc + 1], scale=1.0)
            for ki in range(4):
                for v2 in range(NQ2):
                    ps = pps.tile([128, QC], F32, tag="pp",
                                  name=f"psv{t4}_{ki}_{v2}")
                    for kc in range(KC_C):
                        nc.tensor.matmul(ps[:, :],
                                         xcT[kc][:, ki * 128:(ki + 1) * 128],
                                         wv[kc][:, v2 * QC:(v2 + 1) * QC],
                                         start=(kc == 0),
                                         stop=(kc == KC_C - 1))
                    nc.vector.tensor_tensor(
                        out=v_g[t4][:, ki, v2 * 8:(v2 + 1) * 8, 0:D],
                        in0=ps[:, :].rearrange("p (h d) -> p h d", d=D),
                        in1=bvb[:, v2 * QC:(v2 + 1) * QC].rearrange(
                            "p (h d) -> p h d", d=D),
                        op=OP.add)
                nc.vector.memset(v_g[t4][:, ki, :, D:D + 1], 1.0)

        # ---- query side: LN, transpose, Q projection (oc-outer so that
        # qTc[hp] completes in hp order and attention can start early) ----
        for kc in range(KC_Q):
            nc.gpsimd.dma_start(out=wq[kc][:, :],
                                in_=t["Wq"].ap()[kc * 128:(kc + 1) * 128, :])
        for i in range(NQT):
            ln_tile(t["xq"], xq_bf, i, CQ, 2, 512, gqb, bqb)
        xqT = [[xTq.tile([128, QC], BF16, tag=f"xqT{kc}_{t2}",
                         name=f"xqT{kc}_{t2}") for t2 in range(NQ2)]
               for kc in range(KC_Q)]
        for kc in range(KC_Q):
            for t2 in range(NQ2):
                nc.sync.dma_start(
                    out=xqT[kc][t2][:, :],
                    in_=xq_bf[t2 * QC:(t2 + 1) * QC,
                              kc * 128:(kc + 1) * 128],
                    transpose=True)
        for oc in range(KC_Q):
            for t2 in range(NQ2):
                ps = pps.tile([128, QC], F32, tag="pp", name=f"psq{oc}_{t2}")
                for kc in range(KC_Q):
                    nc.tensor.matmul(ps[:, :],
                                     wq[kc][:, oc * 128:(oc + 1) * 128],
                                     xqT[kc][t2][:, :],
                                     start=(kc == 0), stop=(kc == KC_Q - 1))
                nc.scalar.activation(
                    out=qTc[oc][:, t2 * QC:(t2 + 1) * QC], in_=ps[:, :],
                    func=AF.Identity, bias=bq_cols[:, oc:oc + 1], scale=1.0)

    # =================== phase B: attention + out proj ===================
    with tc.tile_pool(name="wop", bufs=1) as wop, \
         tc.tile_pool(name="rp", bufs=2) as rp, \
         tc.tile_pool(name="op", bufs=1) as op_pool:

        wo = wop.tile([128, KC_Q, CQ], BF16, name="wo")
        attT = wop.tile([128, KC_Q, NQ], BF16, name="attT")
        osb = [op_pool.tile([128, CQ], F32, tag=f"osb{qt}", name=f"osb{qt}")
               for qt in range(NQT)]

        def out_proj_pass(lo_kc, hi_kc, first):
            """Partial out-projection over attT chunks [lo_kc, hi_kc);
            the first pass adds the bias, the last one DMAs out."""
            for qt in range(NQT):
                for cc in range(NQ2):
                    ps = pps.tile([128, QC], F32, tag="pp",
                                  name=f"pso{qt}_{cc}_{lo_kc}")
                    for kc in range(lo_kc, hi_kc):
                        nc.tensor.matmul(
                            ps[:, :],
                            attT[:, kc, qt * 128:(qt + 1) * 128],
                            wo[:, kc, cc * QC:(cc + 1) * QC],
                            start=(kc == lo_kc), stop=(kc == hi_kc - 1))
                    nc.vector.tensor_tensor(
                        out=osb[qt][:, cc * QC:(cc + 1) * QC],
                        in0=ps[:, :],
                        in1=(bob[:, cc * QC:(cc + 1) * QC] if first
                             else osb[qt][:, cc * QC:(cc + 1) * QC]),
                        op=OP.add)
                if not first:
                    nc.sync.dma_start(
                        out=out.ap()[qt * 128:(qt + 1) * 128, :],
                        in_=osb[qt][:, :])
        for kc in range(KC_Q):
            nc.gpsimd.dma_start(out=wo[:, kc, :],
                                in_=t["Wo"].ap()[kc * 128:(kc + 1) * 128, :])

        for hp in range(H // 2):
            atc = {par: rp.tile([65, NQ], BF16, tag=f"atc{par}",
                                name=f"atc{hp}_{par}") for par in range(2)}
            for qh in range(NQ2):
                att = {par: attps.tile([65, QC], F32, tag=f"att{par}",
                                       name=f"att{hp}_{qh}_{par}")
                       for par in range(2)}
                for kt in range(NKT):
                    sc = scps.tile([128, 2, QC], F32, tag="sc",
                                   name=f"sc{hp}_{qh}_{kt}")
                    # the two heads' score matmuls use disjoint PE row
                    # groups (rows 0-63 / 64-127) -> concurrent execution
                    for par in range(2):
                        lo = par * 64
                        nc.tensor.matmul(
                            sc[:, par, :],
                            kTc[hp][kt // 4][lo:lo + 64,
                                             (kt % 4) * 128:(kt % 4 + 1) * 128],
                            qTc[hp][lo:lo + 64, qh * QC:(qh + 1) * QC],
                            start=True, stop=True)
                    e = ep.tile([128, 2, QC], BF16, tag="e",
                                name=f"e{hp}_{qh}_{kt}")
                    nc.scalar.activation(out=e[:, :, :], in_=sc[:, :, :],
                                         func=AF.Exp, scale=SM_SCALE)
                    for par in range(2):
                        h = 2 * hp + par
                        nc.tensor.matmul(
                            att[par][:, :],
                            v_g[kt // 4][:, kt % 4, h, :],
                            e[:, par, :],
                            start=(kt == 0), stop=(kt == NKT - 1))
                for par in range(2):
                    nc.vector.tensor_copy(
                        out=atc[par][:, qh * QC:(qh + 1) * QC],
                        in_=att[par][:, :])
                    h = 2 * hp + par
                    nc.sync.dma_start(
                        out=den_d[0:1,
                                  h * NQ + qh * QC:h * NQ + (qh + 1) * QC],
                        in_=atc[par][64:65, qh * QC:(qh + 1) * QC])
            # batched reciprocal of this pair's 4 denominator rows:
            # [16, 128] uses 16 partitions instead of 1
            dsb = rp.tile([16, 128], F32, tag="dsb", name=f"dsb{hp}")
            nc.gpsimd.dma_start(
                out=dsb[:, :],
                in_=bass.AP(tensor=den_d.tensor,
                            offset=den_d.offset + 2 * hp * NQ,
                            ap=[[128, 16], [1, 128]]))
            drc = rp.tile([16, 128], F32, tag="drc", name=f"drc{hp}")
            nc.vector.reciprocal(out=drc[:, :], in_=dsb[:, :])
            nc.sync.dma_start(
                out=bass.AP(tensor=denr_d.tensor,
                            offset=denr_d.offset + 2 * hp * NQ,
                            ap=[[128, 16], [1, 128]]),
                in_=drc[:, :])
            for par in range(2):
                h = 2 * hp + par
                rb = rp.tile([64, NQ], F32, tag=f"rb{par}", name=f"rb{h}")
                nc.gpsimd.dma_start(
                    out=rb[:, :],
                    in_=bass.AP(tensor=denr_d.tensor,
                                offset=denr_d.offset + h * NQ,
                                ap=[[0, 64], [1, NQ]]))
                if par == 0:
                    nc.vector.tensor_mul(out=attT[0:64, hp, :],
                                         in0=atc[par][0:64, :], in1=rb[:, :])
                else:
                    # odd head: normalize at partitions 0-63, then DMA
                    # shifts it to partitions 64-127 of the attT chunk
                    tm = rp.tile([64, NQ], BF16, tag="tm", name=f"tm{h}")
                    nc.vector.tensor_mul(out=tm[:, :],
                                         in0=atc[par][0:64, :], in1=rb[:, :])
                    nc.sync.dma_start(out=attT[64:128, hp, :], in_=tm[:, :])
            if hp == 3:
                # first half of the out projection runs under the
                # (ACT-bound) attention of head pairs 4-7
                out_proj_pass(0, KC_Q // 2, first=True)

        out_proj_pass(KC_Q // 2, KC_Q, first=False)

    es.close()


def build():
    nc = bass.Bass("TRN2", target_bir_lowering=False, debug=False,
                   num_devices=N_CORES)
    t = {
        "xq": nc.dram_tensor("xq", [NQ, CQ], F32, kind="ExternalInput"),
        "xc": nc.dram_tensor("xc", [NK, CK], F32, kind="ExternalInput"),
        "Wq": nc.dram_tensor("Wq", [CQ, CQ], F32, kind="ExternalInput"),
        "Wk": nc.dram_tensor("Wk", [CK, CQ], F32, kind="ExternalInput"),
        "Wv": nc.dram_tensor("Wv", [CK, CQ], F32, kind="ExternalInput"),
        "Wo": nc.dram_tensor("Wo", [CQ, CQ], F32, kind="ExternalInput"),
        "bq": nc.dram_tensor("bq", [CQ], F32, kind="ExternalInput"),
        "bk": nc.dram_tensor("bk", [CQ], F32, kind="ExternalInput"),
        "bv": nc.dram_tensor("bv", [CQ], F32, kind="ExternalInput"),
        "bo": nc.dram_tensor("bo", [CQ], F32, kind="ExternalInput"),
        "gamma_q": nc.dram_tensor("gamma_q", [CQ], F32, kind="ExternalInput"),
        "beta_q": nc.dram_tensor("beta_q", [CQ], F32, kind="ExternalInput"),
        "gamma_ctx": nc.dram_tensor("gamma_ctx", [CK], F32, kind="ExternalInput"),
        "beta_ctx": nc.dram_tensor("beta_ctx", [CK], F32, kind="ExternalInput"),
    }
    out = nc.dram_tensor("out", [NQ, CQ], F32, kind="ExternalOutput")
    with tile.TileContext(nc) as tc:
        _emit(tc, t, out)
    _split_excess_waits(nc)
    return nc


_NC = None


def _in_maps(inputs):
    q = np.ascontiguousarray(np.asarray(inputs["query_tokens"], dtype=np.float32))
    c = np.ascontiguousarray(np.asarray(inputs["context_tokens"], dtype=np.float32))
    shared = {k: np.ascontiguousarray(np.asarray(inputs[k], dtype=np.float32))
              for k in ("Wq", "Wk", "Wv", "Wo", "bq", "bk", "bv", "bo",
                        "gamma_q", "beta_q", "gamma_ctx", "beta_ctx")}
    maps = []
    for core in range(N_CORES):
        b, half = core // 2, core % 2
        m = dict(shared)
        m["xq"] = np.ascontiguousarray(q[b, half * NQ:(half + 1) * NQ, :])
        m["xc"] = np.ascontiguousarray(c[b])
        maps.append(m)
    return maps


def run_sharded(inputs, **kwargs):
    global _NC
    if _NC is None:
        _NC = build()
    return run_bass_kernel_spmd(_NC, _in_maps(inputs),
                                core_ids=list(range(N_CORES)), **kwargs)


def assemble(res) -> np.ndarray:
    out = np.empty((B, NQ_FULL, CQ), np.float32)
    for core in range(N_CORES):
        b, half = core // 2, core % 2
        out[b, half * NQ:(half + 1) * NQ, :] = res.results[core]["out"]
    return out


def kernel(**inputs) -> np.ndarray:
    return assemble(run_sharded(inputs))


# revision 12
# speedup vs baseline: 1.6917x; 1.1525x over previous
"""Trainium2 Bass kernel for CrossAttention (LN -> QKV proj -> MHA -> out proj).

Sharding: data-parallel over (batch, query-half): 8 shards for B=4.
Each core gets a [1024, 1024] query-token slice and the full [2048, 768]
context for its batch, and produces a [1024, 1024] output slice.

Per-core dataflow (matmul operands bf16, accumulation fp32 in PSUM):
  - LayerNorm in natural [tok, C] layout (DVE bn_stats/bn_aggr); the
    normalized bf16 activations bounce through a DRAM scratch so the
    channel-major transpose runs as a few large DMA-xbar transposes
    ([512 tok, 128 ch] each) instead of many [128,128] ones.
  - Weights are cast fp32->bf16 during the load DMA (SWDGE).
  - Context side (K/V) is projected first so attention can start while
    the query-side projection passes still run under it.
  - Attention per head-pair hp, query-half qh (512 q), context tile kt:
    the two heads' score matmuls have contraction D=64 at disjoint PE
    row groups (partitions 0-63 / 64-127) with identical deps, so they
    execute concurrently in the array; one exp (ACT) covers both heads'
    scores [128, 2*512] straight out of PSUM.  attended accumulates per
    (hp, qh, par) into a [65, 512] psum over all kt, with a ones column
    appended to V so row 64 yields the softmax denominator for free.
  - Denominators of the 4 (head, qh) rows of a pair are batched through
    a DRAM scratch into a [16, 128] tile for one partition-parallel
    fast reciprocal, then broadcast-DMA'd back to [64, 1024] rows for
    the normalize multiply (DVE), off the attention critical path.
  - out = attT^T @ Wo + bo per 128-token tile, DMA out.

PSUM budget: 2 banks proj/out + 2x2 banks scores + 2 banks attended = 8.
SBUF: persistent (qTc/kTc/v_g/attT ~100KB/part) + phase-scoped pools.
"""

import numpy as np

import concourse.bass as bass
import concourse.tile as tile
from concourse import mybir
from concourse.bass_utils import run_bass_kernel_spmd

F32 = mybir.dt.float32
BF16 = mybir.dt.bfloat16
AF = mybir.ActivationFunctionType
OP = mybir.AluOpType

B, NQ_FULL, NK, CQ, CK, H, D = 4, 2048, 2048, 1024, 768, 16, 64
NQ = 1024            # per-core query tokens
N_CORES = 8
EPS = 1e-5
SM_SCALE = 1.0 / np.sqrt(D)  # 0.125

KC_Q = CQ // 128     # 8  channel chunks for CQ
KC_C = CK // 128     # 6  channel chunks for CK
NQT = NQ // 128      # 8  query token tiles
NKT = NK // 128      # 16 context token tiles
QC = 512             # psum free-dim limit (fp32)
T4 = NK // QC        # 4  context 512-token chunks
NQ2 = NQ // QC       # 2  query 512-token chunks


def _split_excess_waits(nc, max_waits=1):
    """walrus in this container accepts at most one sync wait per
    instruction; Tile's kernel-tail drain carries several.  Hoist excess
    waits onto single-wait NOPs that precede the instruction on the same
    engine (absolute sem waits commute, so this is semantics-preserving)."""
    for fn in nc.m.functions:
        for blk in fn.blocks:
            out = []
            dirty = False
            for inst in list(blk.instructions):
                si = inst.sync_info
                if si is not None and len(si.on_wait) > max_waits:
                    waits = list(si.on_wait)
                    for k, w in enumerate(waits[:-max_waits]):
                        nop = mybir.InstNoOp(
                            name=f"wsplit-{inst.name}-{k}", ins=[], outs=[])
                        nop.engine = inst.engine
                        nop.sync_info = mybir.SyncInfo(on_wait=[w], on_update=[])
                        out.append(nop)
                    inst.sync_info = mybir.SyncInfo(
                        on_wait=waits[-max_waits:], on_update=list(si.on_update))
                    dirty = True
                out.append(inst)
            if dirty:
                blk.instructions = out


def _bcast_ap(handle, n_parts, n_free):
    """DRAM [n_free] vector replicated across n_parts partitions."""
    return bass.AP(tensor=handle.ap().tensor, offset=0,
                   ap=[[0, n_parts], [1, n_free]])


def _emit(tc, t, out):
    from contextlib import ExitStack
    nc = tc.nc

    es = ExitStack()
    persist = es.enter_context(tc.tile_pool(name="persist", bufs=1))
    dram = es.enter_context(tc.tile_pool(name="dram", bufs=1, space="DRAM"))

    # ---- tensors live across phases ----
    qTc = [persist.tile([128, NQ], BF16, tag=f"qT{oc}", name=f"qT{oc}")
           for oc in range(KC_Q)]
    kTc = [[persist.tile([128, QC], BF16, tag=f"kT{oc}_{t4}",
                         name=f"kT{oc}_{t4}") for t4 in range(T4)]
           for oc in range(KC_Q)]
    # V with a ones column per head: attended matmul row 64 = sum(exp)
    v_g = [persist.tile([128, 4, H, D + 1], BF16, tag=f"v{g}", name=f"v{g}")
           for g in range(T4)]
    bq_cols = persist.tile([128, KC_Q], F32)
    bk_cols = persist.tile([128, KC_Q], F32)
    bob = persist.tile([128, CQ], F32)
    eps_t = persist.tile([128, 1], F32)

    # DRAM scratch
    xq_bf = dram.tile([NQ, CQ], BF16, name="xq_bf")
    xc_bf = dram.tile([NK, CK], BF16, name="xc_bf")
    den_d = dram.tile([1, H * NQ], BF16, name="den_d")
    denr_d = dram.tile([1, H * NQ], F32, name="denr_d")

    # ---- PSUM pools: 2 (proj/out) + 4 (scores x2bufs) + 2 (attended) = 8
    pps = es.enter_context(tc.tile_pool(name="pps", bufs=2, space="PSUM"))
    scps = es.enter_context(tc.tile_pool(name="scps", bufs=2, space="PSUM"))
    attps = es.enter_context(tc.tile_pool(name="attps", bufs=1, space="PSUM"))

    # e tiles sit on attention's critical path at the phase boundary, so
    # they get addresses disjoint from the phase-A pools below.
    ep = es.enter_context(tc.tile_pool(name="ep", bufs=3))

    nc.vector.memset(eps_t[:, :], EPS)
    nc.sync.dma_start(out=bq_cols[:, :],
                      in_=t["bq"].ap().rearrange("(j p) -> p j", p=128))
    nc.sync.dma_start(out=bk_cols[:, :],
                      in_=t["bk"].ap().rearrange("(j p) -> p j", p=128))


    # =================== phase A: LN + QKV projections ===================
    with tc.tile_pool(name="wp", bufs=1) as wp, \
         tc.tile_pool(name="lnc", bufs=1) as lnc, \
         tc.tile_pool(name="xfp", bufs=2) as xfp, \
         tc.tile_pool(name="stp", bufs=3) as stp, \
         tc.tile_pool(name="bfp", bufs=2) as bfp, \
         tc.tile_pool(name="xTq", bufs=1) as xTq, \
         tc.tile_pool(name="xTc", bufs=1) as xTc:

        # weights: fp32 DRAM -> bf16 SBUF, cast during SWDGE DMA
        wq = [wp.tile([128, CQ], BF16, tag=f"wq{k}", name=f"wq{k}")
              for k in range(KC_Q)]
        wk = [wp.tile([128, CQ], BF16, tag=f"wk{k}", name=f"wk{k}")
              for k in range(KC_C)]
        wv = [wp.tile([128, CQ], BF16, tag=f"wv{k}", name=f"wv{k}")
              for k in range(KC_C)]
        gqb = lnc.tile([128, CQ], F32)
        bqb = lnc.tile([128, CQ], F32)
        gcb = lnc.tile([128, CK], F32)
        bcb = lnc.tile([128, CK], F32)
        bvb = lnc.tile([128, CQ], F32)
        # context LN consts first (needed within ~8us), then K/V weights,
        # then everything whose first use comes later
        nc.gpsimd.dma_start(out=gcb[:, :], in_=_bcast_ap(t["gamma_ctx"], 128, CK))
        nc.gpsimd.dma_start(out=bcb[:, :], in_=_bcast_ap(t["beta_ctx"], 128, CK))
        for kc in range(KC_C):
            nc.gpsimd.dma_start(out=wk[kc][:, :],
                                in_=t["Wk"].ap()[kc * 128:(kc + 1) * 128, :])
            nc.gpsimd.dma_start(out=wv[kc][:, :],
                                in_=t["Wv"].ap()[kc * 128:(kc + 1) * 128, :])
        nc.gpsimd.dma_start(out=bvb[:, :], in_=_bcast_ap(t["bv"], 128, CQ))
        nc.gpsimd.dma_start(out=gqb[:, :], in_=_bcast_ap(t["gamma_q"], 128, CQ))
        nc.gpsimd.dma_start(out=bqb[:, :], in_=_bcast_ap(t["beta_q"], 128, CQ))
        nc.gpsimd.dma_start(out=bob[:, :], in_=_bcast_ap(t["bo"], 128, CQ))

        def ln_tile(x_dram, bf_dram, i, C, n_sub, sub, gb, bb):
            """LN one [128, C] token tile in natural layout; bf16 result
            goes to the DRAM bounce buffer for the batched transpose."""
            xf = xfp.tile([128, CQ], F32, tag="xf", name=f"xf_{i}_{C}")
            nc.scalar.dma_start(out=xf[:, 0:C],
                                in_=x_dram.ap()[i * 128:(i + 1) * 128, :])
            st = stp.tile([128, n_sub, 6], F32, tag="st", name=f"st_{i}_{C}")
            for s in range(n_sub):
                nc.vector.bn_stats(out=st[:, s, :],
                                   in_=xf[:, s * sub:(s + 1) * sub])
            mv = stp.tile([128, 2], F32, tag="mv", name=f"mv_{i}_{C}")
            nc.vector.bn_aggr(out=mv[:, :], in_=st[:, :, :])
            nc.scalar.activation(out=mv[:, 1:2], in_=mv[:, 1:2],
                                 func=AF.Sqrt, bias=eps_t[:, :], scale=1.0)
            nc.vector.reciprocal(out=mv[:, 1:2], in_=mv[:, 1:2])
            nc.vector.tensor_scalar(out=xf[:, 0:C], in0=xf[:, 0:C],
                                    scalar1=mv[:, 0:1], scalar2=mv[:, 1:2],
                                    op0=OP.subtract, op1=OP.mult)
            nc.vector.tensor_mul(out=xf[:, 0:C], in0=xf[:, 0:C], in1=gb[:, :])
            xbf = bfp.tile([128, CQ], BF16, tag="xbf", name=f"xbf_{i}_{C}")
            nc.vector.tensor_add(out=xbf[:, 0:C], in0=xf[:, 0:C], in1=bb[:, :])
            nc.sync.dma_start(out=bf_dram[i * 128:(i + 1) * 128, :],
                              in_=xbf[:, 0:C])

        # ---- context side: LN, transpose, K/V projections ----
        for t4 in range(T4):
            for i in range(4):
                ln_tile(t["xc"], xc_bf, t4 * 4 + i, CK, 3, 256, gcb, bcb)
            xcT = [xTc.tile([128, QC], BF16, tag=f"xcT{kc}",
                            name=f"xcT{kc}_{t4}") for kc in range(KC_C)]
            for kc in range(KC_C):
                nc.sync.dma_start(
                    out=xcT[kc][:, :],
                    in_=xc_bf[t4 * QC:(t4 + 1) * QC,
                              kc * 128:(kc + 1) * 128],
                    transpose=True)
            for oc in range(KC_Q):
                ps = pps.tile([128, QC], F32, tag="pp", name=f"psk{oc}_{t4}")
                for kc in range(KC_C):
                    nc.tensor.matmul(ps[:, :],
                                     wk[kc][:, oc * 128:(oc + 1) * 128],
                                     xcT[kc][:, :],
                                     start=(kc == 0), stop=(kc == KC_C - 1))
                nc.scalar.activation(
                    out=kTc[oc][t4][:, :], in_=ps[:, :], func=AF.Identity,
                    bias=bk_cols[:, oc:oc + 1], scale=1.0)
            for ki in range(4):
                for v2 in range(NQ2):
                    ps = pps.tile([128, QC], F32, tag="pp",
                                  name=f"psv{t4}_{ki}_{v2}")
                    for kc in range(KC_C):
                        nc.tensor.matmul(ps[:, :],
                                         xcT[kc][:, ki * 128:(ki + 1) * 128],
                                         wv[kc][:, v2 * QC:(v2 + 1) * QC],
                                         start=(kc == 0),
                                         stop=(kc == KC_C - 1))
                    nc.vector.tensor_tensor(
                        out=v_g[t4][:, ki, v2 * 8:(v2 + 1) * 8, 0:D],
                        in0=ps[:, :].rearrange("p (h d) -> p h d", d=D),
                        in1=bvb[:, v2 * QC:(v2 + 1) * QC].rearrange(
                            "p (h d) -> p h d", d=D),
                        op=OP.add)
                nc.vector.memset(v_g[t4][:, ki, :, D:D + 1], 1.0)

        # ---- query side: LN, transpose, Q projection (oc-outer so that
        # qTc[hp] completes in hp order and attention can start early) ----
        for kc in range(KC_Q):
            nc.gpsimd.dma_start(out=wq[kc][:, :],
                                in_=t["Wq"].ap()[kc * 128:(kc + 1) * 128, :])
        for i in range(NQT):
            ln_tile(t["xq"], xq_bf, i, CQ, 2, 512, gqb, bqb)
        xqT = [[xTq.tile([128, QC], BF16, tag=f"xqT{kc}_{t2}",
                         name=f"xqT{kc}_{t2}") for t2 in range(NQ2)]
               for kc in range(KC_Q)]
        for kc in range(KC_Q):
            for t2 in range(NQ2):
                nc.sync.dma_start(
                    out=xqT[kc][t2][:, :],
                    in_=xq_bf[t2 * QC:(t2 + 1) * QC,
                              kc * 128:(kc + 1) * 128],
                    transpose=True)
        for oc in range(KC_Q):
            for t2 in range(NQ2):
                ps = pps.tile([128, QC], F32, tag="pp", name=f"psq{oc}_{t2}")
                for kc in range(KC_Q):
                    nc.tensor.matmul(ps[:, :],
                                     wq[kc][:, oc * 128:(oc + 1) * 128],
                                     xqT[kc][t2][:, :],
                                     start=(kc == 0), stop=(kc == KC_Q - 1))
                nc.scalar.activation(
                    out=qTc[oc][:, t2 * QC:(t2 + 1) * QC], in_=ps[:, :],
                    func=AF.Identity, bias=bq_cols[:, oc:oc + 1], scale=1.0)

    # =================== phase B: attention + out proj ===================
    with tc.tile_pool(name="wop", bufs=1) as wop, \
         tc.tile_pool(name="rp", bufs=2) as rp, \
         tc.tile_pool(name="op", bufs=1) as op_pool:

        wo = wop.tile([128, KC_Q, CQ], BF16, name="wo")
        attT = wop.tile([128, KC_Q, NQ], BF16, name="attT")
        osb = [op_pool.tile([128, CQ], F32, tag=f"osb{qt}", name=f"osb{qt}")
               for qt in range(NQT)]

        def out_proj_pass(lo_kc, hi_kc, first):
            """Partial out-projection over attT chunks [lo_kc, hi_kc);
            the first pass adds the bias, the last one DMAs out."""
            for qt in range(NQT):
                for cc in range(NQ2):
                    ps = pps.tile([128, QC], F32, tag="pp",
                                  name=f"pso{qt}_{cc}_{lo_kc}")
                    for kc in range(lo_kc, hi_kc):
                        nc.tensor.matmul(
                            ps[:, :],
                            attT[:, kc, qt * 128:(qt + 1) * 128],
                            wo[:, kc, cc * QC:(cc + 1) * QC],
                            start=(kc == lo_kc), stop=(kc == hi_kc - 1))
                    nc.vector.tensor_tensor(
                        out=osb[qt][:, cc * QC:(cc + 1) * QC],
                        in0=ps[:, :],
                        in1=(bob[:, cc * QC:(cc + 1) * QC] if first
                             else osb[qt][:, cc * QC:(cc + 1) * QC]),
                        op=OP.add)
                if not first:
                    nc.sync.dma_start(
                        out=out.ap()[qt * 128:(qt + 1) * 128, :],
                        in_=osb[qt][:, :])
        for kc in range(KC_Q):
            nc.gpsimd.dma_start(out=wo[:, kc, :],
                                in_=t["Wo"].ap()[kc * 128:(kc + 1) * 128, :])

        for hp in range(H // 2):
            atc = {par: rp.tile([65, NQ], BF16, tag=f"atc{par}",
                                name=f"atc{hp}_{par}") for par in range(2)}
            for qh in range(NQ2):
                att = {par: attps.tile([65, QC], F32, tag=f"att{par}",
                                       name=f"att{hp}_{qh}_{par}")
                       for par in range(2)}
                for kt in range(NKT):
                    sc = scps.tile([128, 2, QC], F32, tag="sc",
                                   name=f"sc{hp}_{qh}_{kt}")
                    # the two heads' score matmuls use disjoint PE row
                    # groups (rows 0-63 / 64-127) -> concurrent execution
                    for par in range(2):
                        lo = par * 64
                        nc.tensor.matmul(
                            sc[:, par, :],
                            kTc[hp][kt // 4][lo:lo + 64,
                                             (kt % 4) * 128:(kt % 4 + 1) * 128],
                            qTc[hp][lo:lo + 64, qh * QC:(qh + 1) * QC],
                            start=True, stop=True)
                    e = ep.tile([128, 2, QC], BF16, tag="e",
                                name=f"e{hp}_{qh}_{kt}")
                    nc.scalar.activation(out=e[:, :, :], in_=sc[:, :, :],
                                         func=AF.Exp, scale=SM_SCALE)
                    for par in range(2):
                        h = 2 * hp + par
                        nc.tensor.matmul(
                            att[par][:, :],
                            v_g[kt // 4][:, kt % 4, h, :],
                            e[:, par, :],
                            start=(kt == 0), stop=(kt == NKT - 1))
                for par in range(2):
                    nc.vector.tensor_copy(
                        out=atc[par][:, qh * QC:(qh + 1) * QC],
                        in_=att[par][:, :])
                    h = 2 * hp + par
                    nc.sync.dma_start(
                        out=den_d[0:1,
                                  h * NQ + qh * QC:h * NQ + (qh + 1) * QC],
                        in_=atc[par][64:65, qh * QC:(qh + 1) * QC])
            # batched reciprocal of this pair's 4 denominator rows:
            # [16, 128] uses 16 partitions instead of 1
            dsb = rp.tile([16, 128], F32, tag="dsb", name=f"dsb{hp}")
            nc.gpsimd.dma_start(
                out=dsb[:, :],
                in_=bass.AP(tensor=den_d.tensor,
                            offset=den_d.offset + 2 * hp * NQ,
                            ap=[[128, 16], [1, 128]]))
            drc = rp.tile([16, 128], F32, tag="drc", name=f"drc{hp}")
            nc.vector.reciprocal(out=drc[:, :], in_=dsb[:, :])
            nc.sync.dma_start(
                out=bass.AP(tensor=denr_d.tensor,
                            offset=denr_d.offset + 2 * hp * NQ,
                            ap=[[128, 16], [1, 128]]),
                in_=drc[:, :])
            for par in range(2):
                h = 2 * hp + par
                rb = rp.tile([64, NQ], F32, tag=f"rb{par}", name=f"rb{h}")
                nc.gpsimd.dma_start(
                    out=rb[:, :],
                    in_=bass.AP(tensor=denr_d.tensor,
                                offset=denr_d.offset + h * NQ,
                                ap=[[0, 64], [1, NQ]]))
                if par == 0:
                    nc.vector.tensor_mul(out=attT[0:64, hp, :],
                                         in0=atc[par][0:64, :], in1=rb[:, :])
                else:
                    # odd head: normalize at partitions 0-63, then DMA
                    # shifts it to partitions 64-127 of the attT chunk
                    tm = rp.tile([64, NQ], BF16, tag="tm", name=f"tm{h}")
                    nc.vector.tensor_mul(out=tm[:, :],
                                         in0=atc[par][0:64, :], in1=rb[:, :])
                    nc.sync.dma_start(out=attT[64:128, hp, :], in_=tm[:, :])
            if hp == 3:
                # first half of the out projection runs under the
                # (ACT-bound) attention of head pairs 4-7
                out_proj_pass(0, KC_Q // 2, first=True)

        out_proj_pass(KC_Q // 2, KC_Q, first=False)

    es.close()


def build():
    nc = bass.Bass("TRN2", target_bir_lowering=False, debug=False,
                   num_devices=N_CORES)
    t = {
        "xq": nc.dram_tensor("xq", [NQ, CQ], F32, kind="ExternalInput"),
        "xc": nc.dram_tensor("xc", [NK, CK], F32, kind="ExternalInput"),
        "Wq": nc.dram_tensor("Wq", [CQ, CQ], F32, kind="ExternalInput"),
        "Wk": nc.dram_tensor("Wk", [CK, CQ], F32, kind="ExternalInput"),
        "Wv": nc.dram_tensor("Wv", [CK, CQ], F32, kind="ExternalInput"),
        "Wo": nc.dram_tensor("Wo", [CQ, CQ], F32, kind="ExternalInput"),
        "bq": nc.dram_tensor("bq", [CQ], F32, kind="ExternalInput"),
        "bk": nc.dram_tensor("bk", [CQ], F32, kind="ExternalInput"),
        "bv": nc.dram_tensor("bv", [CQ], F32, kind="ExternalInput"),
        "bo": nc.dram_tensor("bo", [CQ], F32, kind="ExternalInput"),
        "gamma_q": nc.dram_tensor("gamma_q", [CQ], F32, kind="ExternalInput"),
        "beta_q": nc.dram_tensor("beta_q", [CQ], F32, kind="ExternalInput"),
        "gamma_ctx": nc.dram_tensor("gamma_ctx", [CK], F32, kind="ExternalInput"),
        "beta_ctx": nc.dram_tensor("beta_ctx", [CK], F32, kind="ExternalInput"),
    }
    out = nc.dram_tensor("out", [NQ, CQ], F32, kind="ExternalOutput")
    with tile.TileContext(nc) as tc:
        _emit(tc, t, out)
    _split_excess_waits(nc)
    return nc


_NC = None


def _in_maps(inputs):
    q = np.ascontiguousarray(np.asarray(inputs["query_tokens"], dtype=np.float32))
    c = np.ascontiguousarray(np.asarray(inputs["context_tokens"], dtype=np.float32))
    shared = {k: np.ascontiguousarray(np.asarray(inputs[k], dtype=np.float32))
              for k in ("Wq", "Wk", "Wv", "Wo", "bq", "bk", "bv", "bo",
                        "gamma_q", "beta_q", "gamma_ctx", "beta_ctx")}
    maps = []
    for core in range(N_CORES):
        b, half = core // 2, core % 2
        m = dict(shared)
        m["xq"] = np.ascontiguousarray(q[b, half * NQ:(half + 1) * NQ, :])
        m["xc"] = np.ascontiguousarray(c[b])
        maps.append(m)
    return maps


def run_sharded(inputs, **kwargs):
    global _NC
    if _NC is None:
        _NC = build()
    return run_bass_kernel_spmd(_NC, _in_maps(inputs),
                                core_ids=list(range(N_CORES)), **kwargs)


def assemble(res) -> np.ndarray:
    out = np.empty((B, NQ_FULL, CQ), np.float32)
    for core in range(N_CORES):
        b, half = core // 2, core % 2
        out[b, half * NQ:(half + 1) * NQ, :] = res.results[core]["out"]
    return out


def kernel(**inputs) -> np.ndarray:
    return assemble(run_sharded(inputs))


# revision 15
# speedup vs baseline: 1.7025x; 1.0064x over previous
"""Trainium2 Bass kernel for CrossAttention (LN -> QKV proj -> MHA -> out proj).

Sharding: data-parallel over (batch, query-half): 8 shards for B=4.
Each core gets a [1024, 1024] query-token slice and the full [2048, 768]
context for its batch, and produces a [1024, 1024] output slice.

Per-core dataflow (matmul operands bf16, accumulation fp32 in PSUM):
  - LayerNorm in natural [tok, C] layout (DVE bn_stats/bn_aggr); the
    normalized bf16 activations bounce through a DRAM scratch so the
    channel-major transpose runs as a few large DMA-xbar transposes
    ([512 tok, 128 ch] each) instead of many [128,128] ones.
  - Weights are cast fp32->bf16 during the load DMA (SWDGE).
  - Context side (K/V) is projected first so attention can start while
    the query-side projection passes still run under it.
  - Attention per head-pair hp, query-half qh (512 q), context tile kt:
    the two heads' score matmuls have contraction D=64 at disjoint PE
    row groups (partitions 0-63 / 64-127) with identical deps, so they
    execute concurrently in the array; one exp (ACT) covers both heads'
    scores [128, 2*512] straight out of PSUM.  attended accumulates per
    (hp, qh, par) into a [65, 512] psum over all kt, with a ones column
    appended to V so row 64 yields the softmax denominator for free.
  - Denominators of the 4 (head, qh) rows of a pair are batched through
    a DRAM scratch into a [16, 128] tile for one partition-parallel
    fast reciprocal, then broadcast-DMA'd back to [64, 1024] rows for
    the normalize multiply (DVE), off the attention critical path.
  - out = attT^T @ Wo + bo per 128-token tile, DMA out.

PSUM budget: 2 banks proj/out + 2x2 banks scores + 2 banks attended = 8.
SBUF: persistent (qTc/kTc/v_g/attT ~100KB/part) + phase-scoped pools.
"""

import numpy as np

import concourse.bass as bass
import concourse.tile as tile
from concourse import mybir
from concourse.bass_utils import run_bass_kernel_spmd

F32 = mybir.dt.float32
BF16 = mybir.dt.bfloat16
AF = mybir.ActivationFunctionType
OP = mybir.AluOpType

B, NQ_FULL, NK, CQ, CK, H, D = 4, 2048, 2048, 1024, 768, 16, 64
NQ = 1024            # per-core query tokens
N_CORES = 8
EPS = 1e-5
SM_SCALE = 1.0 / np.sqrt(D)  # 0.125

KC_Q = CQ // 128     # 8  channel chunks for CQ
KC_C = CK // 128     # 6  channel chunks for CK
NQT = NQ // 128      # 8  query token tiles
NKT = NK // 128      # 16 context token tiles
QC = 512             # psum free-dim limit (fp32)
T4 = NK // QC        # 4  context 512-token chunks
NQ2 = NQ // QC       # 2  query 512-token chunks


def _split_excess_waits(nc, max_waits=1):
    """walrus in this container accepts at most one sync wait per
    instruction; Tile's kernel-tail drain carries several.  Hoist excess
    waits onto single-wait NOPs that precede the instruction on the same
    engine (absolute sem waits commute, so this is semantics-preserving)."""
    for fn in nc.m.functions:
        for blk in fn.blocks:
            out = []
            dirty = False
            for inst in list(blk.instructions):
                si = inst.sync_info
                if si is not None and len(si.on_wait) > max_waits:
                    waits = list(si.on_wait)
                    for k, w in enumerate(waits[:-max_waits]):
                        nop = mybir.InstNoOp(
                            name=f"wsplit-{inst.name}-{k}", ins=[], outs=[])
                        nop.engine = inst.engine
                        nop.sync_info = mybir.SyncInfo(on_wait=[w], on_update=[])
                        out.append(nop)
                    inst.sync_info = mybir.SyncInfo(
                        on_wait=waits[-max_waits:], on_update=list(si.on_update))
                    dirty = True
                out.append(inst)
            if dirty:
                blk.instructions = out


def _bcast_ap(handle, n_parts, n_free):
    """DRAM [n_free] vector replicated across n_parts partitions."""
    return bass.AP(tensor=handle.ap().tensor, offset=0,
                   ap=[[0, n_parts], [1, n_free]])


def _emit(tc, t, out):
    from contextlib import ExitStack
    nc = tc.nc

    es = ExitStack()
    persist = es.enter_context(tc.tile_pool(name="persist", bufs=1))
    dram = es.enter_context(tc.tile_pool(name="dram", bufs=1, space="DRAM"))

    # ---- tensors live across phases ----
    qTc = [persist.tile([128, NQ], BF16, tag=f"qT{oc}", name=f"qT{oc}")
           for oc in range(KC_Q)]
    kTc = [[persist.tile([128, QC], BF16, tag=f"kT{oc}_{t4}",
                         name=f"kT{oc}_{t4}") for t4 in range(T4)]
           for oc in range(KC_Q)]
    # V with a ones column per head: attended matmul row 64 = sum(exp)
    v_g = [persist.tile([128, 4, H, D + 1], BF16, tag=f"v{g}", name=f"v{g}")
           for g in range(T4)]
    bq_cols = persist.tile([128, KC_Q], F32)
    bk_cols = persist.tile([128, KC_Q], F32)
    eps_t = persist.tile([128, 1], F32)

    # DRAM scratch
    xq_bf = dram.tile([NQ, CQ], BF16, name="xq_bf")
    xc_bf = dram.tile([NK, CK], BF16, name="xc_bf")
    den_d = dram.tile([1, H * NQ], BF16, name="den_d")
    att_part = dram.tile([H, D + 1, NQ], BF16, name="att_part")
    denr_d = dram.tile([1, H * NQ], F32, name="denr_d")

    # ---- PSUM pools: 2 (proj/out) + 4 (scores x2bufs) + 2 (attended) = 8
    pps = es.enter_context(tc.tile_pool(name="pps", bufs=2, space="PSUM"))
    scps = es.enter_context(tc.tile_pool(name="scps", bufs=2, space="PSUM"))
    attps = es.enter_context(tc.tile_pool(name="attps", bufs=1, space="PSUM"))

    # e tiles sit on attention's critical path at the phase boundary, so
    # they get addresses disjoint from the phase-A pools below.
    ep = es.enter_context(tc.tile_pool(name="ep", bufs=3))
    ppsb = es.enter_context(tc.tile_pool(name="ppsb", bufs=3))

    nc.vector.memset(eps_t[:, :], EPS)
    nc.sync.dma_start(out=bq_cols[:, :],
                      in_=t["bq"].ap().rearrange("(j p) -> p j", p=128))
    nc.sync.dma_start(out=bk_cols[:, :],
                      in_=t["bk"].ap().rearrange("(j p) -> p j", p=128))


    # =================== phase A: LN + QKV projections ===================
    with tc.tile_pool(name="wp", bufs=1) as wp, \
         tc.tile_pool(name="lnc", bufs=1) as lnc, \
         tc.tile_pool(name="xfp", bufs=2) as xfp, \
         tc.tile_pool(name="stp", bufs=3) as stp, \
         tc.tile_pool(name="bfp", bufs=2) as bfp, \
         tc.tile_pool(name="xTq", bufs=1) as xTq, \
         tc.tile_pool(name="xTc", bufs=1) as xTc:

        # weights: fp32 DRAM -> bf16 SBUF, cast during SWDGE DMA
        wq = [wp.tile([128, CQ], BF16, tag=f"wq{k}", name=f"wq{k}")
              for k in range(KC_Q)]
        wk = [wp.tile([128, CQ], BF16, tag=f"wk{k}", name=f"wk{k}")
              for k in range(KC_C)]
        wv = [wp.tile([128, CQ], BF16, tag=f"wv{k}", name=f"wv{k}")
              for k in range(KC_C)]
        gqb = lnc.tile([128, CQ], F32)
        bqb = lnc.tile([128, CQ], F32)
        gcb = lnc.tile([128, CK], F32)
        bcb = lnc.tile([128, CK], F32)
        bvb = lnc.tile([128, CQ], F32)
        # context LN consts first (needed within ~8us), then K/V weights,
        # then everything whose first use comes later
        nc.gpsimd.dma_start(out=gcb[:, :], in_=_bcast_ap(t["gamma_ctx"], 128, CK))
        nc.gpsimd.dma_start(out=bcb[:, :], in_=_bcast_ap(t["beta_ctx"], 128, CK))
        for kc in range(KC_C):
            nc.gpsimd.dma_start(out=wk[kc][:, :],
                                in_=t["Wk"].ap()[kc * 128:(kc + 1) * 128, :])
            nc.gpsimd.dma_start(out=wv[kc][:, :],
                                in_=t["Wv"].ap()[kc * 128:(kc + 1) * 128, :])
        nc.gpsimd.dma_start(out=bvb[:, :], in_=_bcast_ap(t["bv"], 128, CQ))
        nc.gpsimd.dma_start(out=gqb[:, :], in_=_bcast_ap(t["gamma_q"], 128, CQ))
        nc.gpsimd.dma_start(out=bqb[:, :], in_=_bcast_ap(t["beta_q"], 128, CQ))

        def ln_tile(x_dram, bf_dram, i, C, n_sub, sub, gb, bb):
            """LN one [128, C] token tile in natural layout; bf16 result
            goes to the DRAM bounce buffer for the batched transpose."""
            xf = xfp.tile([128, CQ], F32, tag="xf", name=f"xf_{i}_{C}")
            nc.scalar.dma_start(out=xf[:, 0:C],
                                in_=x_dram.ap()[i * 128:(i + 1) * 128, :])
            st = stp.tile([128, n_sub, 6], F32, tag="st", name=f"st_{i}_{C}")
            for s in range(n_sub):
                nc.vector.bn_stats(out=st[:, s, :],
                                   in_=xf[:, s * sub:(s + 1) * sub])
            mv = stp.tile([128, 2], F32, tag="mv", name=f"mv_{i}_{C}")
            nc.vector.bn_aggr(out=mv[:, :], in_=st[:, :, :])
            nc.scalar.activation(out=mv[:, 1:2], in_=mv[:, 1:2],
                                 func=AF.Sqrt, bias=eps_t[:, :], scale=1.0)
            nc.vector.reciprocal(out=mv[:, 1:2], in_=mv[:, 1:2])
            nc.vector.tensor_scalar(out=xf[:, 0:C], in0=xf[:, 0:C],
                                    scalar1=mv[:, 0:1], scalar2=mv[:, 1:2],
                                    op0=OP.subtract, op1=OP.mult)
            nc.vector.tensor_mul(out=xf[:, 0:C], in0=xf[:, 0:C], in1=gb[:, :])
            xbf = bfp.tile([128, CQ], BF16, tag="xbf", name=f"xbf_{i}_{C}")
            nc.vector.tensor_add(out=xbf[:, 0:C], in0=xf[:, 0:C], in1=bb[:, :])
            nc.sync.dma_start(out=bf_dram[i * 128:(i + 1) * 128, :],
                              in_=xbf[:, 0:C])

        # ---- context side: LN, transpose, K/V projections ----
        def context_chunk(t4):
            for i in range(4):
                ln_tile(t["xc"], xc_bf, t4 * 4 + i, CK, 3, 256, gcb, bcb)
            xcT = [xTc.tile([128, QC], BF16, tag=f"xcT{kc}",
                            name=f"xcT{kc}_{t4}") for kc in range(KC_C)]

            for kc in range(KC_C):
                nc.sync.dma_start(
                    out=xcT[kc][:, :],
                    in_=xc_bf[t4 * QC:(t4 + 1) * QC,
                              kc * 128:(kc + 1) * 128],
                    transpose=True)
            for oc in range(KC_Q):
                ps = pps.tile([128, QC], F32, tag="pp", name=f"psk{oc}_{t4}")
                for kc in range(KC_C):
                    nc.tensor.matmul(ps[:, :],
                                     wk[kc][:, oc * 128:(oc + 1) * 128],
                                     xcT[kc][:, :],
                                     start=(kc == 0), stop=(kc == KC_C - 1))
                nc.scalar.activation(
                    out=kTc[oc][t4][:, :], in_=ps[:, :], func=AF.Identity,
                    bias=bk_cols[:, oc:oc + 1], scale=1.0)
            for ki in range(4):
                for v2 in range(NQ2):
                    ps = pps.tile([128, QC], F32, tag="pp",
                                  name=f"psv{t4}_{ki}_{v2}")
                    for kc in range(KC_C):
                        nc.tensor.matmul(ps[:, :],
                                         xcT[kc][:, ki * 128:(ki + 1) * 128],
                                         wv[kc][:, v2 * QC:(v2 + 1) * QC],
                                         start=(kc == 0),
                                         stop=(kc == KC_C - 1))
                    nc.vector.tensor_tensor(
                        out=v_g[t4][:, ki, v2 * 8:(v2 + 1) * 8, 0:D],
                        in0=ps[:, :].rearrange("p (h d) -> p h d", d=D),
                        in1=bvb[:, v2 * QC:(v2 + 1) * QC].rearrange(
                            "p (h d) -> p h d", d=D),
                        op=OP.add)
                nc.vector.memset(v_g[t4][:, ki, :, D:D + 1], 1.0)

        for t4 in range(2):
            context_chunk(t4)

        # ---- query side: LN, transpose, Q projection (oc-outer so that
        # qTc[hp] completes in hp order and attention can start early) ----
        for kc in range(KC_Q):
            nc.gpsimd.dma_start(out=wq[kc][:, :],
                                in_=t["Wq"].ap()[kc * 128:(kc + 1) * 128, :])
        for i in range(NQT):
            ln_tile(t["xq"], xq_bf, i, CQ, 2, 512, gqb, bqb)
        xqT = [[xTq.tile([128, QC], BF16, tag=f"xqT{kc}_{t2}",
                         name=f"xqT{kc}_{t2}") for t2 in range(NQ2)]
               for kc in range(KC_Q)]
        for kc in range(KC_Q):
            for t2 in range(NQ2):
                nc.sync.dma_start(
                    out=xqT[kc][t2][:, :],
                    in_=xq_bf[t2 * QC:(t2 + 1) * QC,
                              kc * 128:(kc + 1) * 128],
                    transpose=True)
        for oc in range(KC_Q):
            for t2 in range(NQ2):
                ps = pps.tile([128, QC], F32, tag="pp", name=f"psq{oc}_{t2}")
                for kc in range(KC_Q):
                    nc.tensor.matmul(ps[:, :],
                                     wq[kc][:, oc * 128:(oc + 1) * 128],
                                     xqT[kc][t2][:, :],
                                     start=(kc == 0), stop=(kc == KC_Q - 1))
                nc.scalar.activation(
                    out=qTc[oc][:, t2 * QC:(t2 + 1) * QC], in_=ps[:, :],
                    func=AF.Identity, bias=bq_cols[:, oc:oc + 1], scale=1.0)

        # ---- attention pass 1 (context tiles 0-7): runs ACT-bound while
        # context chunks 2-3 stream through LN/DMA under it; unnormalized
        # partials (and partial denominators) park in DRAM ----
        for hp in range(H // 2):
            for qh in range(NQ2):
                att = {par: attps.tile([65, QC], F32, tag=f"att{par}",
                                       name=f"a1_{hp}_{qh}_{par}")
                       for par in range(2)}
                for kt in range(NKT // 2):
                    sc = scps.tile([128, 2, QC], F32, tag="sc",
                                   name=f"sc1_{hp}_{qh}_{kt}")
                    for par in range(2):
                        lo = par * 64
                        nc.tensor.matmul(
                            sc[:, par, :],
                            kTc[hp][kt // 4][lo:lo + 64,
                                             (kt % 4) * 128:(kt % 4 + 1) * 128],
                            qTc[hp][lo:lo + 64, qh * QC:(qh + 1) * QC],
                            start=True, stop=True)
                    e = ep.tile([128, 2, QC], BF16, tag="e",
                                name=f"e1_{hp}_{qh}_{kt}")
                    nc.scalar.activation(out=e[:, :, :], in_=sc[:, :, :],
                                         func=AF.Exp, scale=SM_SCALE)
                    for par in range(2):
                        h = 2 * hp + par
                        nc.tensor.matmul(
                            att[par][:, :],
                            v_g[kt // 4][:, kt % 4, h, :],
                            e[:, par, :],
                            start=(kt == 0), stop=(kt == NKT // 2 - 1))
                for par in range(2):
                    h = 2 * hp + par
                    pp = ppsb.tile([D + 1, QC], BF16, tag="pp_sb",
                                   name=f"pp{hp}_{qh}_{par}")
                    nc.vector.tensor_copy(out=pp[:, :], in_=att[par][:, :])
                    nc.sync.dma_start(
                        out=att_part[h, :, qh * QC:(qh + 1) * QC],
                        in_=pp[:, :])

        for t4 in range(2, 4):
            context_chunk(t4)

    # =================== phase B: attention + out proj ===================
    with tc.tile_pool(name="wop", bufs=1) as wop, \
         tc.tile_pool(name="rp", bufs=2) as rp, \
         tc.tile_pool(name="op", bufs=2) as op_pool:

        wo = wop.tile([128, KC_Q, CQ], BF16, name="wo")
        attT = wop.tile([128, KC_Q, NQ], BF16, name="attT")
        bob = wop.tile([128, CQ], F32, name="bob")
        for kc in range(KC_Q):
            nc.gpsimd.dma_start(out=wo[:, kc, :],
                                in_=t["Wo"].ap()[kc * 128:(kc + 1) * 128, :])
        nc.gpsimd.dma_start(out=bob[:, :], in_=_bcast_ap(t["bo"], 128, CQ))

        for hp in range(H // 2):
            atc = {par: rp.tile([65, NQ], BF16, tag=f"atc{par}",
                                name=f"atc{hp}_{par}") for par in range(2)}
            for qh in range(NQ2):
                att = {par: attps.tile([65, QC], F32, tag=f"att{par}",
                                       name=f"att{hp}_{qh}_{par}")
                       for par in range(2)}
                for kt in range(NKT // 2, NKT):
                    sc = scps.tile([128, 2, QC], F32, tag="sc",
                                   name=f"sc2_{hp}_{qh}_{kt}")
                    for par in range(2):
                        lo = par * 64
                        nc.tensor.matmul(
                            sc[:, par, :],
                            kTc[hp][kt // 4][lo:lo + 64,
                                             (kt % 4) * 128:(kt % 4 + 1) * 128],
                            qTc[hp][lo:lo + 64, qh * QC:(qh + 1) * QC],
                            start=True, stop=True)
                    e = ep.tile([128, 2, QC], BF16, tag="e",
                                name=f"e2_{hp}_{qh}_{kt}")
                    nc.scalar.activation(out=e[:, :, :], in_=sc[:, :, :],
                                         func=AF.Exp, scale=SM_SCALE)
                    for par in range(2):
                        h = 2 * hp + par
                        nc.tensor.matmul(
                            att[par][:, :],
                            v_g[kt // 4][:, kt % 4, h, :],
                            e[:, par, :],
                            start=(kt == NKT // 2), stop=(kt == NKT - 1))
                for par in range(2):
                    h = 2 * hp + par
                    pl = ppsb.tile([D + 1, QC], BF16, tag="pl",
                                   name=f"pl{hp}_{qh}_{par}")
                    nc.sync.dma_start(
                        out=pl[:, :],
                        in_=att_part[h, :, qh * QC:(qh + 1) * QC])
                    nc.vector.tensor_tensor(
                        out=atc[par][:, qh * QC:(qh + 1) * QC],
                        in0=att[par][:, :], in1=pl[:, :], op=OP.add)
                    nc.sync.dma_start(
                        out=den_d[0:1,
                                  h * NQ + qh * QC:h * NQ + (qh + 1) * QC],
                        in_=atc[par][64:65, qh * QC:(qh + 1) * QC])
            # batched reciprocal of this pair's 4 denominator rows:
            # [16, 128] uses 16 partitions instead of 1
            dsb = rp.tile([16, 128], F32, tag="dsb", name=f"dsb{hp}")
            nc.gpsimd.dma_start(
                out=dsb[:, :],
                in_=bass.AP(tensor=den_d.tensor,
                            offset=den_d.offset + 2 * hp * NQ,
                            ap=[[128, 16], [1, 128]]))
            drc = rp.tile([16, 128], F32, tag="drc", name=f"drc{hp}")
            nc.vector.reciprocal(out=drc[:, :], in_=dsb[:, :])
            nc.sync.dma_start(
                out=bass.AP(tensor=denr_d.tensor,
                            offset=denr_d.offset + 2 * hp * NQ,
                            ap=[[128, 16], [1, 128]]),
                in_=drc[:, :])
            for par in range(2):
                h = 2 * hp + par
                rb = rp.tile([64, NQ], F32, tag=f"rb{par}", name=f"rb{h}")
                nc.gpsimd.dma_start(
                    out=rb[:, :],
                    in_=bass.AP(tensor=denr_d.tensor,
                                offset=denr_d.offset + h * NQ,
                                ap=[[0, 64], [1, NQ]]))
                if par == 0:
                    nc.vector.tensor_mul(out=attT[0:64, hp, :],
                                         in0=atc[par][0:64, :], in1=rb[:, :])
                else:
                    # odd head: normalize at partitions 0-63, then DMA
                    # shifts it to partitions 64-127 of the attT chunk
                    tm = rp.tile([64, NQ], BF16, tag="tm", name=f"tm{h}")
                    nc.vector.tensor_mul(out=tm[:, :],
                                         in0=atc[par][0:64, :], in1=rb[:, :])
                    nc.sync.dma_start(out=attT[64:128, hp, :], in_=tm[:, :])
        for qt in range(NQT):
            osb = op_pool.tile([128, CQ], F32, tag="osb", name=f"osb{qt}")
            for cc in range(NQ2):
                ps = pps.tile([128, QC], F32, tag="pp", name=f"pso{qt}_{cc}")
                for kc in range(KC_Q):
                    nc.tensor.matmul(
                        ps[:, :],
                        attT[:, kc, qt * 128:(qt + 1) * 128],
                        wo[:, kc, cc * QC:(cc + 1) * QC],
                        start=(kc == 0), stop=(kc == KC_Q - 1))
                nc.vector.tensor_tensor(out=osb[:, cc * QC:(cc + 1) * QC],
                                        in0=ps[:, :],
                                        in1=bob[:, cc * QC:(cc + 1) * QC],
                                        op=OP.add)
            nc.sync.dma_start(out=out.ap()[qt * 128:(qt + 1) * 128, :],
                              in_=osb[:, :])

    es.close()


def build():
    nc = bass.Bass("TRN2", target_bir_lowering=False, debug=False,
                   num_devices=N_CORES)
    t = {
        "xq": nc.dram_tensor("xq", [NQ, CQ], F32, kind="ExternalInput"),
        "xc": nc.dram_tensor("xc", [NK, CK], F32, kind="ExternalInput"),
        "Wq": nc.dram_tensor("Wq", [CQ, CQ], F32, kind="ExternalInput"),
        "Wk": nc.dram_tensor("Wk", [CK, CQ], F32, kind="ExternalInput"),
        "Wv": nc.dram_tensor("Wv", [CK, CQ], F32, kind="ExternalInput"),
        "Wo": nc.dram_tensor("Wo", [CQ, CQ], F32, kind="ExternalInput"),
        "bq": nc.dram_tensor("bq", [CQ], F32, kind="ExternalInput"),
        "bk": nc.dram_tensor("bk", [CQ], F32, kind="ExternalInput"),
        "bv": nc.dram_tensor("bv", [CQ], F32, kind="ExternalInput"),
        "bo": nc.dram_tensor("bo", [CQ], F32, kind="ExternalInput"),
        "gamma_q": nc.dram_tensor("gamma_q", [CQ], F32, kind="ExternalInput"),
        "beta_q": nc.dram_tensor("beta_q", [CQ], F32, kind="ExternalInput"),
        "gamma_ctx": nc.dram_tensor("gamma_ctx", [CK], F32, kind="ExternalInput"),
        "beta_ctx": nc.dram_tensor("beta_ctx", [CK], F32, kind="ExternalInput"),
    }
    out = nc.dram_tensor("out", [NQ, CQ], F32, kind="ExternalOutput")
    with tile.TileContext(nc) as tc:
        _emit(tc, t, out)
    _split_excess_waits(nc)
    return nc


_NC = None


def _in_maps(inputs):
    q = np.ascontiguousarray(np.asarray(inputs["query_tokens"], dtype=np.float32))
    c = np.ascontiguousarray(np.asarray(inputs["context_tokens"], dtype=np.float32))
    shared = {k: np.ascontiguousarray(np.asarray(inputs[k], dtype=np.float32))
              for k in ("Wq", "Wk", "Wv", "Wo", "bq", "bk", "bv", "bo",
                        "gamma_q", "beta_q", "gamma_ctx", "beta_ctx")}
    maps = []
    for core in range(N_CORES):
        b, half = core // 2, core % 2
        m = dict(shared)
        m["xq"] = np.ascontiguousarray(q[b, half * NQ:(half + 1) * NQ, :])
        m["xc"] = np.ascontiguousarray(c[b])
        maps.append(m)
    return maps


def run_sharded(inputs, **kwargs):
    global _NC
    if _NC is None:
        _NC = build()
    return run_bass_kernel_spmd(_NC, _in_maps(inputs),
                                core_ids=list(range(N_CORES)), **kwargs)


def assemble(res) -> np.ndarray:
    out = np.empty((B, NQ_FULL, CQ), np.float32)
    for core in range(N_CORES):
        b, half = core // 2, core % 2
        out[b, half * NQ:(half + 1) * NQ, :] = res.results[core]["out"]
    return out


def kernel(**inputs) -> np.ndarray:
    return assemble(run_sharded(inputs))


# revision 18
# speedup vs baseline: 1.7398x; 1.0219x over previous
"""Trainium2 Bass kernel for CrossAttention (LN -> QKV proj -> MHA -> out proj).

Sharding: data-parallel over (batch, query-half): 8 shards for B=4.
Each core gets a [1024, 1024] query-token slice and the full [2048, 768]
context for its batch, and produces a [1024, 1024] output slice.

Per-core dataflow (matmul operands bf16, accumulation fp32 in PSUM):
  - LayerNorm in natural [tok, C] layout (DVE bn_stats/bn_aggr); the
    normalized bf16 activations bounce through a DRAM scratch so the
    channel-major transpose runs as a few large DMA-xbar transposes
    ([512 tok, 128 ch] each) instead of many [128,128] ones.
  - Weights are cast fp32->bf16 during the load DMA (SWDGE).
  - Context side (K/V) is projected first so attention can start while
    the query-side projection passes still run under it.
  - Attention per head-pair hp, query-half qh (512 q), context tile kt:
    the two heads' score matmuls have contraction D=64 at disjoint PE
    row groups (partitions 0-63 / 64-127) with identical deps, so they
    execute concurrently in the array; one exp (ACT) covers both heads'
    scores [128, 2*512] straight out of PSUM.  attended accumulates per
    (hp, qh, par) into a [65, 512] psum over all kt, with a ones column
    appended to V so row 64 yields the softmax denominator for free.
  - Denominators of the 4 (head, qh) rows of a pair are batched through
    a DRAM scratch into a [16, 128] tile for one partition-parallel
    fast reciprocal, then broadcast-DMA'd back to [64, 1024] rows for
    the normalize multiply (DVE), off the attention critical path.
  - out = attT^T @ Wo + bo per 128-token tile, DMA out.

PSUM budget: 2 banks proj/out + 2x2 banks scores + 2 banks attended = 8.
SBUF: persistent (qTc/kTc/v_g/attT ~100KB/part) + phase-scoped pools.
"""

import numpy as np

import concourse.bass as bass
import concourse.tile as tile
from concourse import mybir
from concourse.bass_utils import run_bass_kernel_spmd

F32 = mybir.dt.float32
BF16 = mybir.dt.bfloat16
AF = mybir.ActivationFunctionType
OP = mybir.AluOpType

B, NQ_FULL, NK, CQ, CK, H, D = 4, 2048, 2048, 1024, 768, 16, 64
NQ = 1024            # per-core query tokens
N_CORES = 8
EPS = 1e-5
SM_SCALE = 1.0 / np.sqrt(D)  # 0.125

KC_Q = CQ // 128     # 8  channel chunks for CQ
KC_C = CK // 128     # 6  channel chunks for CK
NQT = NQ // 128      # 8  query token tiles
NKT = NK // 128      # 16 context token tiles
QC = 512             # psum free-dim limit (fp32)
T4 = NK // QC        # 4  context 512-token chunks
NQ2 = NQ // QC       # 2  query 512-token chunks


def _split_excess_waits(nc, max_waits=1):
    """walrus in this container accepts at most one sync wait per
    instruction; Tile's kernel-tail drain carries several.  Hoist excess
    waits onto single-wait NOPs that precede the instruction on the same
    engine (absolute sem waits commute, so this is semantics-preserving)."""
    for fn in nc.m.functions:
        for blk in fn.blocks:
            out = []
            dirty = False
            for inst in list(blk.instructions):
                si = inst.sync_info
                if si is not None and len(si.on_wait) > max_waits:
                    waits = list(si.on_wait)
                    for k, w in enumerate(waits[:-max_waits]):
                        nop = mybir.InstNoOp(
                            name=f"wsplit-{inst.name}-{k}", ins=[], outs=[])
                        nop.engine = inst.engine
                        nop.sync_info = mybir.SyncInfo(on_wait=[w], on_update=[])
                        out.append(nop)
                    inst.sync_info = mybir.SyncInfo(
                        on_wait=waits[-max_waits:], on_update=list(si.on_update))
                    dirty = True
                out.append(inst)
            if dirty:
                blk.instructions = out


def _bcast_ap(handle, n_parts, n_free):
    """DRAM [n_free] vector replicated across n_parts partitions."""
    return bass.AP(tensor=handle.ap().tensor, offset=0,
                   ap=[[0, n_parts], [1, n_free]])


def _emit(tc, t, out):
    from contextlib import ExitStack
    nc = tc.nc

    es = ExitStack()
    persist = es.enter_context(tc.tile_pool(name="persist", bufs=1))
    dram = es.enter_context(tc.tile_pool(name="dram", bufs=1, space="DRAM"))

    # ---- tensors live across phases ----
    qTc = [persist.tile([128, NQ], BF16, tag=f"qT{oc}", name=f"qT{oc}")
           for oc in range(KC_Q)]
    kTc = [[persist.tile([128, QC], BF16, tag=f"kT{oc}_{t4}",
                         name=f"kT{oc}_{t4}") for t4 in range(T4)]
           for oc in range(KC_Q)]
    # V with a ones column per head: attended matmul row 64 = sum(exp)
    v_g = [persist.tile([128, 4, H, D + 1], BF16, tag=f"v{g}", name=f"v{g}")
           for g in range(T4)]
    bq_cols = persist.tile([128, KC_Q], F32)
    bk_cols = persist.tile([128, KC_Q], F32)
    eps_t = persist.tile([128, 1], F32)

    # DRAM scratch
    xq_bf = dram.tile([NQ, CQ], BF16, name="xq_bf")
    xc_bf = dram.tile([NK, CK], BF16, name="xc_bf")
    den_d = dram.tile([1, H * NQ], BF16, name="den_d")
    att_part = dram.tile([H, D + 1, NQ], BF16, name="att_part")
    denr_d = dram.tile([1, H * NQ], F32, name="denr_d")

    # ---- PSUM pools: 2 (proj/out) + 4 (scores x2bufs) + 2 (attended) = 8
    pps = es.enter_context(tc.tile_pool(name="pps", bufs=2, space="PSUM"))
    scps = es.enter_context(tc.tile_pool(name="scps", bufs=2, space="PSUM"))
    attps = es.enter_context(tc.tile_pool(name="attps", bufs=1, space="PSUM"))

    # e tiles sit on attention's critical path at the phase boundary, so
    # they get addresses disjoint from the phase-A pools below.
    ep = es.enter_context(tc.tile_pool(name="ep", bufs=3))
    ppsb = es.enter_context(tc.tile_pool(name="ppsb", bufs=3))

    nc.vector.memset(eps_t[:, :], EPS)
    nc.sync.dma_start(out=bq_cols[:, :],
                      in_=t["bq"].ap().rearrange("(j p) -> p j", p=128))
    nc.sync.dma_start(out=bk_cols[:, :],
                      in_=t["bk"].ap().rearrange("(j p) -> p j", p=128))


    # =================== phase A: LN + QKV projections ===================
    with tc.tile_pool(name="wp", bufs=1) as wp, \
         tc.tile_pool(name="lnc", bufs=1) as lnc, \
         tc.tile_pool(name="xfp", bufs=2) as xfp, \
         tc.tile_pool(name="stp", bufs=3) as stp, \
         tc.tile_pool(name="bfp", bufs=2) as bfp, \
         tc.tile_pool(name="xTq", bufs=1) as xTq, \
         tc.tile_pool(name="xTc", bufs=1) as xTc:

        # weights: fp32 DRAM -> bf16 SBUF, cast during SWDGE DMA
        wq = [wp.tile([128, CQ], BF16, tag=f"wq{k}", name=f"wq{k}")
              for k in range(KC_Q)]
        wk = [wp.tile([128, CQ], BF16, tag=f"wk{k}", name=f"wk{k}")
              for k in range(KC_C)]
        wv = [wp.tile([128, CQ], BF16, tag=f"wv{k}", name=f"wv{k}")
              for k in range(KC_C)]
        gqb = lnc.tile([128, CQ], F32)
        bqb = lnc.tile([128, CQ], F32)
        gcb = lnc.tile([128, CK], F32)
        bcb = lnc.tile([128, CK], F32)
        bvb = lnc.tile([128, CQ], F32)
        # context LN consts first (needed within ~8us), then K/V weights,
        # then everything whose first use comes later
        nc.gpsimd.dma_start(out=gcb[:, :], in_=_bcast_ap(t["gamma_ctx"], 128, CK))
        nc.gpsimd.dma_start(out=bcb[:, :], in_=_bcast_ap(t["beta_ctx"], 128, CK))
        for kc in range(KC_C):
            nc.gpsimd.dma_start(out=wk[kc][:, :],
                                in_=t["Wk"].ap()[kc * 128:(kc + 1) * 128, :])
            nc.gpsimd.dma_start(out=wv[kc][:, :],
                                in_=t["Wv"].ap()[kc * 128:(kc + 1) * 128, :])
        nc.gpsimd.dma_start(out=bvb[:, :], in_=_bcast_ap(t["bv"], 128, CQ))
        nc.gpsimd.dma_start(out=gqb[:, :], in_=_bcast_ap(t["gamma_q"], 128, CQ))
        nc.gpsimd.dma_start(out=bqb[:, :], in_=_bcast_ap(t["beta_q"], 128, CQ))

        def ln_tile(x_dram, bf_dram, i, C, n_sub, sub, gb, bb):
            """LN one [128, C] token tile in natural layout; bf16 result
            goes to the DRAM bounce buffer for the batched transpose."""
            xf = xfp.tile([128, CQ], F32, tag="xf", name=f"xf_{i}_{C}")
            nc.scalar.dma_start(out=xf[:, 0:C],
                                in_=x_dram.ap()[i * 128:(i + 1) * 128, :])
            st = stp.tile([128, n_sub, 6], F32, tag="st", name=f"st_{i}_{C}")
            for s in range(n_sub):
                nc.vector.bn_stats(out=st[:, s, :],
                                   in_=xf[:, s * sub:(s + 1) * sub])
            mv = stp.tile([128, 2], F32, tag="mv", name=f"mv_{i}_{C}")
            nc.vector.bn_aggr(out=mv[:, :], in_=st[:, :, :])
            nc.scalar.activation(out=mv[:, 1:2], in_=mv[:, 1:2],
                                 func=AF.Sqrt, bias=eps_t[:, :], scale=1.0)
            nc.vector.reciprocal(out=mv[:, 1:2], in_=mv[:, 1:2])
            nc.vector.tensor_scalar(out=xf[:, 0:C], in0=xf[:, 0:C],
                                    scalar1=mv[:, 0:1], scalar2=mv[:, 1:2],
                                    op0=OP.subtract, op1=OP.mult)
            nc.vector.tensor_mul(out=xf[:, 0:C], in0=xf[:, 0:C], in1=gb[:, :])
            xbf = bfp.tile([128, CQ], BF16, tag="xbf", name=f"xbf_{i}_{C}")
            nc.vector.tensor_add(out=xbf[:, 0:C], in0=xf[:, 0:C], in1=bb[:, :])
            nc.sync.dma_start(out=bf_dram[i * 128:(i + 1) * 128, :],
                              in_=xbf[:, 0:C])

        # ---- context side: LN, transpose, K/V projections ----
        def context_ln(t4):
            for i in range(4):
                ln_tile(t["xc"], xc_bf, t4 * 4 + i, CK, 3, 256, gcb, bcb)

        def context_proj(t4):
            xcT = [xTc.tile([128, QC], BF16, tag=f"xcT{kc}",
                            name=f"xcT{kc}_{t4}") for kc in range(KC_C)]

            for kc in range(KC_C):
                nc.sync.dma_start(
                    out=xcT[kc][:, :],
                    in_=xc_bf[t4 * QC:(t4 + 1) * QC,
                              kc * 128:(kc + 1) * 128],
                    transpose=True)
            for oc in range(KC_Q):
                ps = pps.tile([128, QC], F32, tag="pp", name=f"psk{oc}_{t4}")
                for kc in range(KC_C):
                    nc.tensor.matmul(ps[:, :],
                                     wk[kc][:, oc * 128:(oc + 1) * 128],
                                     xcT[kc][:, :],
                                     start=(kc == 0), stop=(kc == KC_C - 1))
                nc.scalar.activation(
                    out=kTc[oc][t4][:, :], in_=ps[:, :], func=AF.Identity,
                    bias=bk_cols[:, oc:oc + 1], scale=1.0)
            for ki in range(4):
                for v2 in range(NQ2):
                    ps = pps.tile([128, QC], F32, tag="pp",
                                  name=f"psv{t4}_{ki}_{v2}")
                    for kc in range(KC_C):
                        nc.tensor.matmul(ps[:, :],
                                         xcT[kc][:, ki * 128:(ki + 1) * 128],
                                         wv[kc][:, v2 * QC:(v2 + 1) * QC],
                                         start=(kc == 0),
                                         stop=(kc == KC_C - 1))
                    nc.vector.tensor_tensor(
                        out=v_g[t4][:, ki, v2 * 8:(v2 + 1) * 8, 0:D],
                        in0=ps[:, :].rearrange("p (h d) -> p h d", d=D),
                        in1=bvb[:, v2 * QC:(v2 + 1) * QC].rearrange(
                            "p (h d) -> p h d", d=D),
                        op=OP.add)
                nc.vector.memset(v_g[t4][:, ki, :, D:D + 1], 1.0)

        for t4 in range(2):
            context_ln(t4)
            context_proj(t4)

        # ---- query side: LN, transpose, Q projection (oc-outer so that
        # qTc[hp] completes in hp order and attention can start early) ----
        for kc in range(KC_Q):
            nc.gpsimd.dma_start(out=wq[kc][:, :],
                                in_=t["Wq"].ap()[kc * 128:(kc + 1) * 128, :])
        for i in range(NQT):
            ln_tile(t["xq"], xq_bf, i, CQ, 2, 512, gqb, bqb)
        xqT = [[xTq.tile([128, QC], BF16, tag=f"xqT{kc}_{t2}",
                         name=f"xqT{kc}_{t2}") for t2 in range(NQ2)]
               for kc in range(KC_Q)]
        for kc in range(KC_Q):
            for t2 in range(NQ2):
                nc.sync.dma_start(
                    out=xqT[kc][t2][:, :],
                    in_=xq_bf[t2 * QC:(t2 + 1) * QC,
                              kc * 128:(kc + 1) * 128],
                    transpose=True)
        for oc in range(KC_Q):
            for t2 in range(NQ2):
                ps = pps.tile([128, QC], F32, tag="pp", name=f"psq{oc}_{t2}")
                for kc in range(KC_Q):
                    nc.tensor.matmul(ps[:, :],
                                     wq[kc][:, oc * 128:(oc + 1) * 128],
                                     xqT[kc][t2][:, :],
                                     start=(kc == 0), stop=(kc == KC_Q - 1))
                nc.scalar.activation(
                    out=qTc[oc][:, t2 * QC:(t2 + 1) * QC], in_=ps[:, :],
                    func=AF.Identity, bias=bq_cols[:, oc:oc + 1], scale=1.0)

        # LN of context chunks 2-3 runs up front so no ACT table switch
        # (Sqrt vs Exp set) lands inside the attention exp stream
        for t4 in range(2, 4):
            context_ln(t4)

        # ---- attention pass 1 (context tiles 0-7): runs ACT-bound while
        # context chunks 2-3 stream through LN/DMA under it; unnormalized
        # partials (and partial denominators) park in DRAM ----
        for hp in range(H // 2):
            for qh in range(NQ2):
                att = {par: attps.tile([65, QC], F32, tag=f"att{par}",
                                       name=f"a1_{hp}_{qh}_{par}")
                       for par in range(2)}
                for kt in range(NKT // 2):
                    sc = scps.tile([128, 2, QC], F32, tag="sc",
                                   name=f"sc1_{hp}_{qh}_{kt}")
                    for par in range(2):
                        lo = par * 64
                        nc.tensor.matmul(
                            sc[:, par, :],
                            kTc[hp][kt // 4][lo:lo + 64,
                                             (kt % 4) * 128:(kt % 4 + 1) * 128],
                            qTc[hp][lo:lo + 64, qh * QC:(qh + 1) * QC],
                            start=True, stop=True)
                    e = ep.tile([128, 2, QC], BF16, tag="e",
                                name=f"e1_{hp}_{qh}_{kt}")
                    nc.scalar.activation(out=e[:, :, :], in_=sc[:, :, :],
                                         func=AF.Exp, scale=SM_SCALE)
                    for par in range(2):
                        h = 2 * hp + par
                        nc.tensor.matmul(
                            att[par][:, :],
                            v_g[kt // 4][:, kt % 4, h, :],
                            e[:, par, :],
                            start=(kt == 0), stop=(kt == NKT // 2 - 1))
                for par in range(2):
                    h = 2 * hp + par
                    pp = ppsb.tile([D + 1, QC], BF16, tag="pp_sb",
                                   name=f"pp{hp}_{qh}_{par}")
                    nc.vector.tensor_copy(out=pp[:, :], in_=att[par][:, :])
                    nc.sync.dma_start(
                        out=att_part[h, :, qh * QC:(qh + 1) * QC],
                        in_=pp[:, :])

        for t4 in range(2, 4):
            context_proj(t4)

    # =================== phase B: attention + out proj ===================
    with tc.tile_pool(name="wop", bufs=1) as wop, \
         tc.tile_pool(name="rp", bufs=2) as rp, \
         tc.tile_pool(name="op", bufs=2) as op_pool:

        wo = wop.tile([128, KC_Q, CQ], BF16, name="wo")
        attT = wop.tile([128, KC_Q, NQ], BF16, name="attT")
        bob = wop.tile([128, CQ], F32, name="bob")
        for kc in range(KC_Q):
            nc.gpsimd.dma_start(out=wo[:, kc, :],
                                in_=t["Wo"].ap()[kc * 128:(kc + 1) * 128, :])
        nc.gpsimd.dma_start(out=bob[:, :], in_=_bcast_ap(t["bo"], 128, CQ))

        for hp in range(H // 2):
            atc = {par: rp.tile([65, NQ], BF16, tag=f"atc{par}",
                                name=f"atc{hp}_{par}") for par in range(2)}
            for qh in range(NQ2):
                att = {par: attps.tile([65, QC], F32, tag=f"att{par}",
                                       name=f"att{hp}_{qh}_{par}")
                       for par in range(2)}
                for kt in range(NKT // 2, NKT):
                    sc = scps.tile([128, 2, QC], F32, tag="sc",
                                   name=f"sc2_{hp}_{qh}_{kt}")
                    for par in range(2):
                        lo = par * 64
                        nc.tensor.matmul(
                            sc[:, par, :],
                            kTc[hp][kt // 4][lo:lo + 64,
                                             (kt % 4) * 128:(kt % 4 + 1) * 128],
                            qTc[hp][lo:lo + 64, qh * QC:(qh + 1) * QC],
                            start=True, stop=True)
                    e = ep.tile([128, 2, QC], BF16, tag="e",
                                name=f"e2_{hp}_{qh}_{kt}")
                    nc.scalar.activation(out=e[:, :, :], in_=sc[:, :, :],
                                         func=AF.Exp, scale=SM_SCALE)
                    for par in range(2):
                        h = 2 * hp + par
                        nc.tensor.matmul(
                            att[par][:, :],
                            v_g[kt // 4][:, kt % 4, h, :],
                            e[:, par, :],
                            start=(kt == NKT // 2), stop=(kt == NKT - 1))
                for par in range(2):
                    h = 2 * hp + par
                    pl = ppsb.tile([D + 1, QC], BF16, tag="pl",
                                   name=f"pl{hp}_{qh}_{par}")
                    nc.sync.dma_start(
                        out=pl[:, :],
                        in_=att_part[h, :, qh * QC:(qh + 1) * QC])
                    nc.vector.tensor_tensor(
                        out=atc[par][:, qh * QC:(qh + 1) * QC],
                        in0=att[par][:, :], in1=pl[:, :], op=OP.add)
                    nc.sync.dma_start(
                        out=den_d[0:1,
                                  h * NQ + qh * QC:h * NQ + (qh + 1) * QC],
                        in_=atc[par][64:65, qh * QC:(qh + 1) * QC])
            # batched reciprocal of this pair's 4 denominator rows:
            # [16, 128] uses 16 partitions instead of 1
            dsb = rp.tile([16, 128], F32, tag="dsb", name=f"dsb{hp}")
            nc.gpsimd.dma_start(
                out=dsb[:, :],
                in_=bass.AP(tensor=den_d.tensor,
                            offset=den_d.offset + 2 * hp * NQ,
                            ap=[[128, 16], [1, 128]]))
            drc = rp.tile([16, 128], F32, tag="drc", name=f"drc{hp}")
            nc.vector.reciprocal(out=drc[:, :], in_=dsb[:, :])
            nc.sync.dma_start(
                out=bass.AP(tensor=denr_d.tensor,
                            offset=denr_d.offset + 2 * hp * NQ,
                            ap=[[128, 16], [1, 128]]),
                in_=drc[:, :])
            for par in range(2):
                h = 2 * hp + par
                rb = rp.tile([64, NQ], F32, tag=f"rb{par}", name=f"rb{h}")
                nc.gpsimd.dma_start(
                    out=rb[:, :],
                    in_=bass.AP(tensor=denr_d.tensor,
                                offset=denr_d.offset + h * NQ,
                                ap=[[0, 64], [1, NQ]]))
                if par == 0:
                    nc.vector.tensor_mul(out=attT[0:64, hp, :],
                                         in0=atc[par][0:64, :], in1=rb[:, :])
                else:
                    # odd head: normalize at partitions 0-63, then DMA
                    # shifts it to partitions 64-127 of the attT chunk
                    tm = rp.tile([64, NQ], BF16, tag="tm", name=f"tm{h}")
                    nc.vector.tensor_mul(out=tm[:, :],
                                         in0=atc[par][0:64, :], in1=rb[:, :])
                    nc.sync.dma_start(out=attT[64:128, hp, :], in_=tm[:, :])
        for qt in range(NQT):
            osb = op_pool.tile([128, CQ], F32, tag="osb", name=f"osb{qt}")
            for cc in range(NQ2):
                ps = pps.tile([128, QC], F32, tag="pp", name=f"pso{qt}_{cc}")
                for kc in range(KC_Q):
                    nc.tensor.matmul(
                        ps[:, :],
                        attT[:, kc, qt * 128:(qt + 1) * 128],
                        wo[:, kc, cc * QC:(cc + 1) * QC],
                        start=(kc == 0), stop=(kc == KC_Q - 1))
                nc.vector.tensor_tensor(out=osb[:, cc * QC:(cc + 1) * QC],
                                        in0=ps[:, :],
                                        in1=bob[:, cc * QC:(cc + 1) * QC],
                                        op=OP.add)
            nc.sync.dma_start(out=out.ap()[qt * 128:(qt + 1) * 128, :],
                              in_=osb[:, :])

    es.close()


def build():
    nc = bass.Bass("TRN2", target_bir_lowering=False, debug=False,
                   num_devices=N_CORES)
    t = {
        "xq": nc.dram_tensor("xq", [NQ, CQ], F32, kind="ExternalInput"),
        "xc": nc.dram_tensor("xc", [NK, CK], F32, kind="ExternalInput"),
        "Wq": nc.dram_tensor("Wq", [CQ, CQ], F32, kind="ExternalInput"),
        "Wk": nc.dram_tensor("Wk", [CK, CQ], F32, kind="ExternalInput"),
        "Wv": nc.dram_tensor("Wv", [CK, CQ], F32, kind="ExternalInput"),
        "Wo": nc.dram_tensor("Wo", [CQ, CQ], F32, kind="ExternalInput"),
        "bq": nc.dram_tensor("bq", [CQ], F32, kind="ExternalInput"),
        "bk": nc.dram_tensor("bk", [CQ], F32, kind="ExternalInput"),
        "bv": nc.dram_tensor("bv", [CQ], F32, kind="ExternalInput"),
        "bo": nc.dram_tensor("bo", [CQ], F32, kind="ExternalInput"),
        "gamma_q": nc.dram_tensor("gamma_q", [CQ], F32, kind="ExternalInput"),
        "beta_q": nc.dram_tensor("beta_q", [CQ], F32, kind="ExternalInput"),
        "gamma_ctx": nc.dram_tensor("gamma_ctx", [CK], F32, kind="ExternalInput"),
        "beta_ctx": nc.dram_tensor("beta_ctx", [CK], F32, kind="ExternalInput"),
    }
    out = nc.dram_tensor("out", [NQ, CQ], F32, kind="ExternalOutput")
    with tile.TileContext(nc) as tc:
        _emit(tc, t, out)
    _split_excess_waits(nc)
    return nc


_NC = None


def _in_maps(inputs):
    q = np.ascontiguousarray(np.asarray(inputs["query_tokens"], dtype=np.float32))
    c = np.ascontiguousarray(np.asarray(inputs["context_tokens"], dtype=np.float32))
    shared = {k: np.ascontiguousarray(np.asarray(inputs[k], dtype=np.float32))
              for k in ("Wq", "Wk", "Wv", "Wo", "bq", "bk", "bv", "bo",
                        "gamma_q", "beta_q", "gamma_ctx", "beta_ctx")}
    maps = []
    for core in range(N_CORES):
        b, half = core // 2, core % 2
        m = dict(shared)
        m["xq"] = np.ascontiguousarray(q[b, half * NQ:(half + 1) * NQ, :])
        m["xc"] = np.ascontiguousarray(c[b])
        maps.append(m)
    return maps


def run_sharded(inputs, **kwargs):
    global _NC
    if _NC is None:
        _NC = build()
    return run_bass_kernel_spmd(_NC, _in_maps(inputs),
                                core_ids=list(range(N_CORES)), **kwargs)


def assemble(res) -> np.ndarray:
    out = np.empty((B, NQ_FULL, CQ), np.float32)
    for core in range(N_CORES):
        b, half = core // 2, core % 2
        out[b, half * NQ:(half + 1) * NQ, :] = res.results[core]["out"]
    return out


def kernel(**inputs) -> np.ndarray:
    return assemble(run_sharded(inputs))
